# revision 3
# baseline (speedup 1.0000x reference)
"""Multi-head causal attention (B=4, S=2048, D=1024, H=16, RoPE) on 8 TRN2 cores.

v2: bf16 compute, token-major PV with ones-column denominators, stream-shuffle
RoPE (pair-interleaved head dims), wide softmax exps, cross-stage interleaving.

Sharding: core = (batch b, head-group g of 8 heads).  Each core computes
qkv projection for its (b, g), RoPE, causal attention, and a partial
out-projection (contraction over its 512 head-dims).  Host sums the two
partials per batch.

Device layouts (per core):
  qk_sb[mo] [128, S]  feature-major bf16; mo 0..3 = q head pairs, 4..7 = k.
                      rows: head-local u*64 + 2*i + p  <->  orig dim i + 32p
                      (pair-interleaved so RoPE rotate-half is an adjacent-pair
                      partition swap, done by stream_shuffle within quadrants)
  v_sb[j]   [128, 520] token-major bf16, 65 cols/head: 64 dims + ones column
                      that makes the P@V matmul accumulate softmax denominators
  st        [t, q]    scores transposed (k-major) so they feed PV as lhsT
  o_ps      [q, 65]   per-(head,q-chunk) attention out + denominator column
  oT_sb[hp] [128, S]  transposed normalized outputs (PE transpose), out-proj lhsT
"""

import math

import numpy as np
import ml_dtypes

import concourse.bass as bass
import concourse.bacc as bacc
import concourse.mybir as mybir
from concourse import tile
from concourse.bass_utils import run_bass_kernel_spmd

AF = mybir.ActivationFunctionType
ALU = mybir.AluOpType
F32 = mybir.dt.float32
BF16 = mybir.dt.bfloat16

N_HEADS = 16
THETA = 10000.0
D = 1024
HD = 64
HL = 8          # heads per core
VW = HD + 1     # v cols per head (64 dims + ones)
QB = 512        # query block / stage-1 token block
TT = 128        # key/value tile
BF = ml_dtypes.bfloat16

SWAP_MASK = []
for _i in range(16):
    SWAP_MASK += [2 * _i + 1, 2 * _i]


def _host_constants(S):
    """RoPE tables (pair-interleaved rows), sign vector, triangle mask."""
    half = HD // 2
    inv = 1.0 / (THETA ** (np.arange(half, dtype=np.float64) / half))
    t = np.arange(S, dtype=np.float64)
    ang = inv[:, None] * t[None, :]                      # [32, S]
    cos = np.cos(ang)
    sin = np.sin(ang)
    # rows: u*64 + 2i + p  -> freq i (independent of u, p)
    cosr = np.repeat(cos, 2, axis=0)                     # [64, S]
    sinr = np.repeat(sin, 2, axis=0)
    ropeC = np.tile(cosr, (2, 1)).astype(BF)             # [128, S]
    sinT = np.tile(sinr, (2, 1)).astype(BF)
    sig = np.tile(np.array([-1.0, 1.0]), 64).astype(np.float32)[:, None]  # [128,1]
    p = np.arange(128)[:, None]
    c = np.arange(128)[None, :]
    trimask = (c >= p).astype(BF)                        # [128, 128]
    ident = np.eye(128, dtype=BF)
    return ropeC, sinT, sig, trimask, ident


def build_nc(S=2048, debug=False):
    nc = bacc.Bacc("TRN2", target_bir_lowering=False, debug=False)

    xT = nc.dram_tensor("xT", [D, S], BF16, kind="ExternalInput").ap()
    wqkT = nc.dram_tensor("wqkT", [D, 2 * HL * HD], BF16, kind="ExternalInput").ap()
    wvT = nc.dram_tensor("wvT", [D, HL * HD], BF16, kind="ExternalInput").ap()
    woutT = nc.dram_tensor("woutT", [HL * HD, D], BF16, kind="ExternalInput").ap()
    outp = nc.dram_tensor("outp", [S, D], F32, kind="ExternalOutput").ap()
    if debug:
        qk_dbg = nc.dram_tensor("qk_dbg", [8 * 128, S], BF16, kind="ExternalOutput").ap()
        v_dbg = nc.dram_tensor("v_dbg", [16 * 128, HL * VW], BF16, kind="ExternalOutput").ap()
        oT_dbg = nc.dram_tensor("oT_dbg", [4 * 128, S], BF16, kind="ExternalOutput").ap()

    ropeC_np, sinT_np, sig_np, trimask_np, ident_np = _host_constants(S)
    tabs_np = np.concatenate([ropeC_np, sinT_np, trimask_np, ident_np], axis=1)
    tabs_d = nc.inline_tensor(tabs_np, "tabs").ap()
    sig_d = nc.inline_tensor(sig_np, "sig").ap()
    identf_d = nc.inline_tensor(ident_np.astype(np.float32), "identf").ap()

    KD = D // 128        # 8 contraction tiles
    nQB = S // QB        # 4
    nMT = S // TT        # 16

    with tile.TileContext(nc) as tc:
        with (
            tc.tile_pool(name="qk", bufs=1) as qk_pool,
            tc.tile_pool(name="vres", bufs=1) as v_pool,
            tc.tile_pool(name="osb", bufs=1) as o_pool,
            tc.tile_pool(name="wqk", bufs=1) as wqk_pool,
            tc.tile_pool(name="wv", bufs=1) as wv_pool,
            tc.tile_pool(name="wout", bufs=1) as wout_pool,
            tc.tile_pool(name="tabs", bufs=1) as tab_pool,
            tc.tile_pool(name="xs", bufs=2) as x_pool,
            tc.tile_pool(name="rope", bufs=3) as rope_pool,
            tc.tile_pool(name="pt", bufs=3) as pt_pool,
            tc.tile_pool(name="opair", bufs=3) as opair_pool,
            tc.tile_pool(name="rec", bufs=4) as rec_pool,
            tc.tile_pool(name="outs", bufs=4) as out_pool,
            tc.tile_pool(name="ps_half", bufs=2, space="PSUM") as half_pool,
            tc.tile_pool(name="ps_st", bufs=2, space="PSUM") as st_pool,
            tc.tile_pool(name="ps_o", bufs=2, space="PSUM") as o_ps_pool,
        ):
            qk_sb = [qk_pool.tile([128, S], BF16, tag=f"qk{i}", name=f"qk{i}")
                     for i in range(8)]
            v_sb = [v_pool.tile([128, HL * VW], BF16, tag=f"v{i}", name=f"v{i}")
                    for i in range(nMT)]
            oT_sb = [o_pool.tile([128, S], BF16, tag=f"oT{i}", name=f"oT{i}")
                     for i in range(4)]
            wqk_big = wqk_pool.tile([128, KD * 128 * 8], BF16, tag="wqk",
                                    name="wqk_big")
            wv_big = wv_pool.tile([128, KD * HL * HD], BF16, tag="wv",
                                  name="wv_big")
            wout_big = wout_pool.tile([128, 4 * D], BF16, tag="wo",
                                      name="wout_big")
            wqk_sb = [wqk_big[:, k * 1024:(k + 1) * 1024] for k in range(KD)]
            wv_sb = [wv_big[:, k * 512:(k + 1) * 512] for k in range(KD)]
            wout_sb = [wout_big[:, i * D:(i + 1) * D] for i in range(4)]
            tabs_sb = tab_pool.tile([128, 2 * S + 256], BF16, tag="tabs",
                                    name="tabs")
            ropeC_sb = tabs_sb[:, 0:S]
            sinT_sb = tabs_sb[:, S:2 * S]
            trimask_sb = tabs_sb[:, 2 * S:2 * S + 128]
            ident_sb = tabs_sb[:, 2 * S + 128:2 * S + 256]
            sig_sb = tab_pool.tile([128, 1], F32, tag="sig", name="sig")
            identf_sb = tab_pool.tile([128, 128], F32, tag="identf",
                                      name="identf")

            # ---------------- prologue DMAs (one big DMA per tensor) -------
            def load_x(nb):
                tk = slice(nb * QB, (nb + 1) * QB)
                xt = x_pool.tile([128, KD * QB], BF16, tag="xt", name="xt")
                nc.sync.dma_start(
                    xt[:].rearrange("p (k c) -> p k c", k=KD),
                    xT[:, tk].rearrange("(k p) c -> p k c", p=128))
                return [xt[:, k * QB:(k + 1) * QB] for k in range(KD)]

            xt0 = x_pool.tile([128, KD * QB], BF16, tag="xt", name="xt")
            for lo in range(0, KD, 2):
                nc.sync.dma_start(
                    wqk_big[:, lo * 1024:(lo + 2) * 1024].rearrange(
                        "p (k c) -> p k c", k=2),
                    wqkT[lo * 128:(lo + 2) * 128, :].rearrange(
                        "(k p) c -> p k c", p=128))
                nc.sync.dma_start(
                    xt0[:, lo * QB:(lo + 2) * QB].rearrange(
                        "p (k c) -> p k c", k=2),
                    xT[lo * 128:(lo + 2) * 128, 0:QB].rearrange(
                        "(k p) c -> p k c", p=128))
            xts0 = [xt0[:, k * QB:(k + 1) * QB] for k in range(KD)]
            nc.sync.dma_start(tabs_sb[:], tabs_d[:])
            nc.sync.dma_start(sig_sb[:], sig_d[:])
            nc.sync.dma_start(identf_sb[:], identf_d[:])
            nc.sync.dma_start(
                wv_big[:].rearrange("p (k c) -> p k c", k=KD),
                wvT.rearrange("(k p) c -> p k c", p=128))
            nc.sync.dma_start(
                wout_big[:].rearrange("p (k c) -> p k c", k=4),
                woutT.rearrange("(k p) c -> p k c", p=128))

            xts_cur = xts0

            # ---------------- stage pieces ----------------
            st_half = {"tile": None}

            def deep_ps(mo):
                """Prologue-only: 6-deep PSUM ring borrowing idle st banks."""
                sel = mo % 3
                if sel < 2:
                    if sel == 0:
                        st_half["tile"] = st_pool.tile(
                            [128, 1024], F32, tag="st", name="ps_qk_st")
                        return st_half["tile"][:, 0:QB]
                    return st_half["tile"][:, QB:2 * QB]
                return half_pool.tile([128, QB], F32, tag="half", name="ps_qk")

            def s1_qk(nb, mo, xts):
                """q/k projection tile mo for token block nb, with RoPE."""
                tok = slice(nb * QB, (nb + 1) * QB)
                ps = half_pool.tile([128, QB], F32, tag="half", name="ps_qk")
                for k in range(KD):
                    nc.tensor.matmul(
                        ps[:],
                        lhsT=wqk_sb[k][:, mo * 128:(mo + 1) * 128],
                        rhs=xts[k][:],
                        start=(k == 0), stop=(k == KD - 1),
                    )
                # RoPE: qk = ps*cos + pairswap(ps)*sig*sin
                swp = rope_pool.tile([128, QB], F32, tag="swp", name="swp")
                nc.vector.stream_shuffle(swp[:], ps[:], SWAP_MASK)
                cq = rope_pool.tile([128, QB], BF16, tag="cq", name="cq")
                nc.vector.scalar_tensor_tensor(
                    cq[:], ps[:], 1.0, ropeC_sb[:, tok],
                    op0=ALU.mult, op1=ALU.mult,
                )
                tm = rope_pool.tile([128, QB], BF16, tag="tm", name="tm")
                nc.vector.scalar_tensor_tensor(
                    tm[:], swp[:], sig_sb[:, 0:1], sinT_sb[:, tok],
                    op0=ALU.mult, op1=ALU.mult,
                )
                nc.gpsimd.tensor_tensor(
                    qk_sb[mo][:, tok], cq[:], tm[:], op=ALU.add)

            def s1_v(nb, mt, xts):
                """v projection for token tile nb*4+mt (token-major + ones)."""
                pv = half_pool.tile([128, QB], F32, tag="half", name="ps_v")
                xsl = slice(mt * 128, (mt + 1) * 128)
                for k in range(KD):
                    nc.tensor.matmul(
                        pv[:],
                        lhsT=xts[k][:, xsl],
                        rhs=wv_sb[k][:],
                        start=(k == 0), stop=(k == KD - 1),
                    )
                vt = v_sb[nb * 4 + mt]
                vdst = vt[:].rearrange("p (h c) -> p h c", h=HL)[:, :, 0:HD]
                nc.vector.tensor_copy(vdst, pv[:].rearrange("p (h c) -> p h c", h=HL))
                ones_ap = vt[:].rearrange("p (h c) -> p h c", h=HL)[:, :, HD]
                nc.vector.memset(ones_ap, 1.0)

            fillers = []       # slow queue: spread over the phase
            fast_fillers = []  # fast queue: one per pair until drained
            quota = {"acc": 0.0, "rate": 1.0}

            def pop_filler():
                if fast_fillers:
                    fast_fillers.pop(0)()
                    return
                quota["acc"] += quota["rate"]
                while fillers and quota["acc"] >= 1.0:
                    quota["acc"] -= 1.0
                    fillers.pop(0)()

            def s2_head(h, qb):
                """Causal attention for head h, query block qb."""
                hp, parity = h // 2, h % 2
                rbase = 64 * parity
                qt = qk_sb[hp]
                kt = qk_sb[4 + hp]
                qsl0 = qb * QB
                o_ps = o_ps_pool.tile([128, 4 * VW], F32, tag="ops", name="o_ps")
                njp = 2 * qb + 2

                def issue_st(jp):
                    st = st_pool.tile([128, 1024], F32, tag="st", name="st")
                    cc0 = max(2 * jp - 4 * qb, 0) * TT
                    for half in (0, 1):
                        j = 2 * jp + half
                        nc.tensor.matmul(
                            st[:, half * QB + cc0:(half + 1) * QB],
                            lhsT=kt[rbase:rbase + HD, j * TT:(j + 1) * TT],
                            rhs=qt[rbase:rbase + HD, qsl0 + cc0:qsl0 + QB],
                            start=True, stop=True,
                        )
                    return st, cc0

                st_cur = issue_st(0)
                for jp in range(njp):
                    st, cc0 = st_cur
                    pt = pt_pool.tile([128, 1024], BF16, tag="pt", name="pt")
                    nc.scalar.activation(pt[:, cc0:1024], st[:, cc0:1024],
                                         AF.Exp, scale=1.0 / math.sqrt(HD))
                    st_cur = issue_st(jp + 1) if jp + 1 < njp else None
                    pop_filler()
                    for half in (0, 1):
                        j = 2 * jp + half
                        oi = j - 4 * qb
                        if oi >= 0:
                            dsl = slice(half * QB + oi * TT,
                                        half * QB + (oi + 1) * TT)
                            nc.vector.tensor_tensor(
                                pt[:, dsl], pt[:, dsl], trimask_sb[:],
                                op=ALU.mult)
                    for half in (0, 1):
                        j = 2 * jp + half
                        oi = j - 4 * qb
                        for qc in range(max(oi, 0), 4):
                            nc.tensor.matmul(
                                o_ps[:, qc * VW:(qc + 1) * VW],
                                lhsT=pt[:, half * QB + qc * TT:
                                        half * QB + (qc + 1) * TT],
                                rhs=v_sb[j][:, VW * h:VW * h + VW],
                                start=(j == 0 and qc == 0),
                                stop=(j == 4 * qb + qc),
                            )
                # normalize: o / denom, written into o_pair tiles (bf16)
                rec = rec_pool.tile([128, 4], F32, tag="rec", name="rec")
                dens = o_ps[:].rearrange("p (q c) -> p q c", q=4)[:, :, HD]
                nc.vector.reciprocal(rec[:], dens)
                return o_ps, rec

            opair_tiles = {}

            def s2_norm(h, qb, o_ps, rec):
                hp, parity = h // 2, h % 2
                key = (hp, qb)
                if key not in opair_tiles:
                    if qb < 3:
                        opair_tiles[key] = opair_pool.tile(
                            [128, QB], BF16, tag="opair", name="opair")
                    else:
                        opair_tiles[key] = opair_pool.tile(
                            [128, QB], F32, tag="opair3", name="opair3")
                opt = opair_tiles[key]
                for qc in range(4):
                    nc.vector.tensor_scalar(
                        opt[:, qc * TT + 64 * parity:qc * TT + 64 * parity + HD],
                        o_ps[:, qc * VW:qc * VW + HD],
                        rec[:, qc:qc + 1], None, op0=ALU.mult)

            def s2_transpose(hp, qb):
                """Transpose the finished o_pair row into oT_sb[hp].

                qb<3 uses the xbar DMA transpose; its consumers run a phase
                later, far beyond the transpose's completion.  qb==3 feeds
                the epilogue within ~1us, and the xbar transpose's
                completion semaphore can fire before all 16 engine chunks
                land, so the epilogue uses PE transposes with exact
                semaphore tracking instead.
                """
                opt = opair_tiles.pop((hp, qb))
                if qb < 3:
                    dst = oT_sb[hp][:, qb * QB:(qb + 1) * QB].rearrange(
                        "p (b q) -> p b q", b=4)
                    nc.sync.dma_start(dst, opt[:], transpose=True)
                    return
                for qc in range(4):
                    qtile = 4 * qb + qc
                    tp_t = half_pool.tile([128, QB], F32, tag="half",
                                          name="tp")
                    nc.tensor.transpose(
                        tp_t[:, 0:128], opt[:, qc * TT:(qc + 1) * TT],
                        identf_sb[:])
                    nc.vector.tensor_copy(
                        oT_sb[hp][:, qtile * TT:(qtile + 1) * TT],
                        tp_t[:, 0:128])

            ot_tiles = {}

            def s3_group(mtt, ib, epi=False):
                tsl = slice(mtt * 128, (mtt + 1) * 128)
                if epi:
                    pot = st_pool.tile([128, 1024], F32, tag="st", name="po_e")
                    po = pot[:, 0:QB]
                else:
                    po = half_pool.tile([128, QB], F32, tag="half", name="po")
                for hp in range(4):
                    nc.tensor.matmul(
                        po[:],
                        lhsT=oT_sb[hp][:, tsl],
                        rhs=wout_sb[hp][:, ib * 512:(ib + 1) * 512],
                        start=(hp == 0), stop=(hp == 3),
                    )
                if mtt not in ot_tiles:
                    ot_tiles[mtt] = out_pool.tile([128, D], F32, tag="ot",
                                                  name="ot")
                ot = ot_tiles[mtt]
                nc.vector.tensor_copy(ot[:, ib * 512:(ib + 1) * 512], po[:])
                if ib == 1:
                    del ot_tiles[mtt]
                    nc.sync.dma_start(outp[tsl, :], ot[:])

            # ---------------- schedule ----------------
            # stage1 block 0 fully first (prologue)
            for mo in range(8):
                s1_qk(0, mo, xts_cur)
            for mt in range(4):
                s1_v(0, mt, xts_cur)

            def F(fn, *args):
                return lambda: fn(*args)

            s3_todo = []   # deferred stage-3 groups from previous qb
            for qb in range(nQB):
                xts_next = load_x(qb + 1) if qb + 1 < nQB else None
                for hp in range(4):
                    for parity in (0, 1):
                        h = 2 * hp + parity
                        o_ps, rec = s2_head(h, qb)
                        s2_norm(h, qb, o_ps, rec)
                    s2_transpose(hp, qb)
                    if xts_next is not None:
                        s1_qk(qb + 1, 2 * hp, xts_next)
                        s1_qk(qb + 1, 2 * hp + 1, xts_next)
                        if hp % 2 == 1:
                            s1_v(qb + 1, hp - 1, xts_next)
                            s1_v(qb + 1, hp, xts_next)
                    n_s3 = {0: 0, 1: 1, 2: 2, 3: 4}[qb]
                    for _ in range(n_s3):
                        if s3_todo:
                            s3_group(*s3_todo.pop(0))
                for mtt in range(4 * qb, 4 * qb + 4):
                    for ib in range(2):
                        s3_todo.append((mtt, ib))
            while s3_todo:
                mtt, ib = s3_todo.pop(0)
                s3_group(mtt, ib)
            if debug:
                for i in range(8):
                    nc.sync.dma_start(qk_dbg[i * 128:(i + 1) * 128, :], qk_sb[i][:])
                for i in range(16):
                    nc.sync.dma_start(v_dbg[i * 128:(i + 1) * 128, :], v_sb[i][:])
                for i in range(4):
                    nc.sync.dma_start(oT_dbg[i * 128:(i + 1) * 128, :], oT_sb[i][:])

    nc.compile()
    return nc


# ---------------------------------------------------------------------------
# host side
# ---------------------------------------------------------------------------

_cache = {}


def _get_nc(S):
    if S not in _cache:
        _cache[S] = build_nc(S)
    return _cache[S]


def _rope_perm():
    """Row permutation within a 64-dim head: r = 2i+p  <-  i + 32p."""
    perm = np.empty(64, dtype=np.int64)
    for i in range(32):
        for p in (0, 1):
            perm[2 * i + p] = i + 32 * p
    return perm


def _shard_weights(w_qkv, w_out, g):
    """Per-head-group weight shards in device layouts (bf16)."""
    w_qkv = np.asarray(w_qkv, dtype=np.float32)
    w_out = np.asarray(w_out, dtype=np.float32)
    perm = _rope_perm()
    rows = []
    for part in range(2):           # 0: q, 1: k
        base = part * D
        for hl in range(HL):
            h_glob = g * HL + hl
            blk = w_qkv[base + h_glob * HD: base + (h_glob + 1) * HD]
            rows.append(blk[perm])
    wqk = np.concatenate(rows, axis=0)                 # [1024, 1024]
    wqkT = np.ascontiguousarray(wqk.T).astype(BF)      # [D, 1024]

    r = slice(2 * D + g * 512, 2 * D + (g + 1) * 512)
    wv = w_qkv[r]                                      # [512, 1024]
    wvT = np.ascontiguousarray(wv.T).astype(BF)        # [D, 512]

    woutT = np.ascontiguousarray(
        w_out.T[g * 512:(g + 1) * 512]).astype(BF)     # [512, 1024]
    return wqkT, wvT, woutT


def kernel(x, w_qkv, w_out):
    x = np.asarray(x, dtype=np.float32)
    B, S, _D = x.shape
    assert _D == D
    nc = _get_nc(S)

    shards = [_shard_weights(w_qkv, w_out, g) for g in range(2)]
    in_maps = []
    for core in range(8):
        b, g = core // 2, core % 2
        wqkT, wvT, woutT = shards[g]
        in_maps.append({
            "xT": np.ascontiguousarray(x[b].T).astype(BF),
            "wqkT": wqkT,
            "wvT": wvT,
            "woutT": woutT,
        })
    res = run_bass_kernel_spmd(nc, in_maps, list(range(8)))
    out = np.empty((B, S, D), dtype=np.float32)
    for b in range(B):
        out[b] = res.results[2 * b]["outp"] + res.results[2 * b + 1]["outp"]
    return out


# revision 4
# speedup vs baseline: 1.0037x; 1.0037x over previous
"""Multi-head causal attention (B=4, S=2048, D=1024, H=16, RoPE) on 8 TRN2 cores.

v2: bf16 compute, token-major PV with ones-column denominators, stream-shuffle
RoPE (pair-interleaved head dims), wide softmax exps, cross-stage interleaving.

Sharding: core = (batch b, head-group g of 8 heads).  Each core computes
qkv projection for its (b, g), RoPE, causal attention, and a partial
out-projection (contraction over its 512 head-dims).  Host sums the two
partials per batch.

Device layouts (per core):
  qk_sb[mo] [128, S]  feature-major bf16; mo 0..3 = q head pairs, 4..7 = k.
                      rows: head-local u*64 + 2*i + p  <->  orig dim i + 32p
                      (pair-interleaved so RoPE rotate-half is an adjacent-pair
                      partition swap, done by stream_shuffle within quadrants)
  v_sb[j]   [128, 520] token-major bf16, 65 cols/head: 64 dims + ones column
                      that makes the P@V matmul accumulate softmax denominators
  st        [t, q]    scores transposed (k-major) so they feed PV as lhsT
  o_ps      [q, 65]   per-(head,q-chunk) attention out + denominator column
  oT_sb[hp] [128, S]  transposed normalized outputs (PE transpose), out-proj lhsT
"""

import math

import numpy as np
import ml_dtypes

import concourse.bass as bass
import concourse.bacc as bacc
import concourse.mybir as mybir
from concourse import tile
from concourse.bass_utils import run_bass_kernel_spmd

AF = mybir.ActivationFunctionType
ALU = mybir.AluOpType
F32 = mybir.dt.float32
BF16 = mybir.dt.bfloat16

N_HEADS = 16
THETA = 10000.0
D = 1024
HD = 64
HL = 8          # heads per core
VW = HD + 1     # v cols per head (64 dims + ones)
QB = 512        # query block / stage-1 token block
TT = 128        # key/value tile
BF = ml_dtypes.bfloat16

SWAP_MASK = []
for _i in range(16):
    SWAP_MASK += [2 * _i + 1, 2 * _i]


def _host_constants(S):
    """RoPE tables (pair-interleaved rows), sign vector, triangle mask."""
    half = HD // 2
    inv = 1.0 / (THETA ** (np.arange(half, dtype=np.float64) / half))
    t = np.arange(S, dtype=np.float64)
    ang = inv[:, None] * t[None, :]                      # [32, S]
    cos = np.cos(ang)
    sin = np.sin(ang)
    # rows: u*64 + 2i + p  -> freq i (independent of u, p)
    cosr = np.repeat(cos, 2, axis=0)                     # [64, S]
    sinr = np.repeat(sin, 2, axis=0)
    ropeC = np.tile(cosr, (2, 1)).astype(BF)             # [128, S]
    sinT = np.tile(sinr, (2, 1)).astype(BF)
    sig = np.tile(np.array([-1.0, 1.0]), 64).astype(np.float32)[:, None]  # [128,1]
    p = np.arange(128)[:, None]
    c = np.arange(128)[None, :]
    trimask = (c >= p).astype(BF)                        # [128, 128]
    ident = np.eye(128, dtype=BF)
    return ropeC, sinT, sig, trimask, ident


def build_nc(S=2048, debug=False):
    nc = bacc.Bacc("TRN2", target_bir_lowering=False, debug=False)

    xT = nc.dram_tensor("xT", [D, S], BF16, kind="ExternalInput").ap()
    wqkT = nc.dram_tensor("wqkT", [D, 2 * HL * HD], BF16, kind="ExternalInput").ap()
    wvT = nc.dram_tensor("wvT", [D, HL * HD], BF16, kind="ExternalInput").ap()
    woutT = nc.dram_tensor("woutT", [HL * HD, D], BF16, kind="ExternalInput").ap()
    outp = nc.dram_tensor("outp", [S, D], F32, kind="ExternalOutput").ap()
    if debug:
        qk_dbg = nc.dram_tensor("qk_dbg", [8 * 128, S], BF16, kind="ExternalOutput").ap()
        v_dbg = nc.dram_tensor("v_dbg", [16 * 128, HL * VW], BF16, kind="ExternalOutput").ap()
        oT_dbg = nc.dram_tensor("oT_dbg", [4 * 128, S], BF16, kind="ExternalOutput").ap()

    ropeC_np, sinT_np, sig_np, trimask_np, ident_np = _host_constants(S)
    tabs_np = np.concatenate([ropeC_np, sinT_np, trimask_np, ident_np], axis=1)
    tabs_d = nc.inline_tensor(tabs_np, "tabs").ap()
    sig_d = nc.inline_tensor(sig_np, "sig").ap()
    identf_d = nc.inline_tensor(ident_np.astype(np.float32), "identf").ap()

    KD = D // 128        # 8 contraction tiles
    nQB = S // QB        # 4
    nMT = S // TT        # 16

    with tile.TileContext(nc) as tc:
        with (
            tc.tile_pool(name="qk", bufs=1) as qk_pool,
            tc.tile_pool(name="vres", bufs=1) as v_pool,
            tc.tile_pool(name="osb", bufs=1) as o_pool,
            tc.tile_pool(name="wqk", bufs=1) as wqk_pool,
            tc.tile_pool(name="wv", bufs=1) as wv_pool,
            tc.tile_pool(name="wout", bufs=1) as wout_pool,
            tc.tile_pool(name="tabs", bufs=1) as tab_pool,
            tc.tile_pool(name="xs", bufs=2) as x_pool,
            tc.tile_pool(name="rope", bufs=3) as rope_pool,
            tc.tile_pool(name="pt", bufs=3) as pt_pool,
            tc.tile_pool(name="opair", bufs=3) as opair_pool,
            tc.tile_pool(name="rec", bufs=4) as rec_pool,
            tc.tile_pool(name="outs", bufs=4) as out_pool,
            tc.tile_pool(name="ps_half", bufs=2, space="PSUM") as half_pool,
            tc.tile_pool(name="ps_st", bufs=2, space="PSUM") as st_pool,
            tc.tile_pool(name="ps_o", bufs=2, space="PSUM") as o_ps_pool,
        ):
            qk_sb = [qk_pool.tile([128, S], BF16, tag=f"qk{i}", name=f"qk{i}")
                     for i in range(8)]
            v_sb = [v_pool.tile([128, HL * VW], BF16, tag=f"v{i}", name=f"v{i}")
                    for i in range(nMT)]
            oT_sb = [o_pool.tile([128, S], BF16, tag=f"oT{i}", name=f"oT{i}")
                     for i in range(4)]
            wqk_big = wqk_pool.tile([128, KD * 128 * 8], BF16, tag="wqk",
                                    name="wqk_big")
            wv_big = wv_pool.tile([128, KD * HL * HD], BF16, tag="wv",
                                  name="wv_big")
            wout_big = wout_pool.tile([128, 4 * D], BF16, tag="wo",
                                      name="wout_big")
            wqk_sb = [wqk_big[:, k * 1024:(k + 1) * 1024] for k in range(KD)]
            wv_sb = [wv_big[:, k * 512:(k + 1) * 512] for k in range(KD)]
            wout_sb = [wout_big[:, i * D:(i + 1) * D] for i in range(4)]
            tabs_sb = tab_pool.tile([128, 2 * S + 256], BF16, tag="tabs",
                                    name="tabs")
            ropeC_sb = tabs_sb[:, 0:S]
            sinT_sb = tabs_sb[:, S:2 * S]
            trimask_sb = tabs_sb[:, 2 * S:2 * S + 128]
            ident_sb = tabs_sb[:, 2 * S + 128:2 * S + 256]
            sig_sb = tab_pool.tile([128, 1], F32, tag="sig", name="sig")
            identf_sb = tab_pool.tile([128, 128], F32, tag="identf",
                                      name="identf")

            # ---------------- prologue DMAs (one big DMA per tensor) -------
            def load_x(nb):
                tk = slice(nb * QB, (nb + 1) * QB)
                xt = x_pool.tile([128, KD * QB], BF16, tag="xt", name="xt")
                nc.sync.dma_start(
                    xt[:].rearrange("p (k c) -> p k c", k=KD),
                    xT[:, tk].rearrange("(k p) c -> p k c", p=128))
                return [xt[:, k * QB:(k + 1) * QB] for k in range(KD)]

            xt0 = x_pool.tile([128, KD * QB], BF16, tag="xt", name="xt")
            for lo in range(0, KD, 2):
                nc.sync.dma_start(
                    wqk_big[:, lo * 1024:(lo + 2) * 1024].rearrange(
                        "p (k c) -> p k c", k=2),
                    wqkT[lo * 128:(lo + 2) * 128, :].rearrange(
                        "(k p) c -> p k c", p=128))
                nc.sync.dma_start(
                    xt0[:, lo * QB:(lo + 2) * QB].rearrange(
                        "p (k c) -> p k c", k=2),
                    xT[lo * 128:(lo + 2) * 128, 0:QB].rearrange(
                        "(k p) c -> p k c", p=128))
            xts0 = [xt0[:, k * QB:(k + 1) * QB] for k in range(KD)]
            nc.sync.dma_start(tabs_sb[:], tabs_d[:])
            nc.sync.dma_start(sig_sb[:], sig_d[:])
            nc.sync.dma_start(identf_sb[:], identf_d[:])
            nc.sync.dma_start(
                wv_big[:].rearrange("p (k c) -> p k c", k=KD),
                wvT.rearrange("(k p) c -> p k c", p=128))
            nc.sync.dma_start(
                wout_big[:].rearrange("p (k c) -> p k c", k=4),
                woutT.rearrange("(k p) c -> p k c", p=128))

            xts_cur = xts0

            # ---------------- stage pieces ----------------
            st_half = {"tile": None}

            def deep_ps(mo):
                """Prologue-only: 6-deep PSUM ring borrowing idle st banks."""
                sel = mo % 3
                if sel < 2:
                    if sel == 0:
                        st_half["tile"] = st_pool.tile(
                            [128, 1024], F32, tag="st", name="ps_qk_st")
                        return st_half["tile"][:, 0:QB]
                    return st_half["tile"][:, QB:2 * QB]
                return half_pool.tile([128, QB], F32, tag="half", name="ps_qk")

            def s1_qk(nb, mo, xts):
                """q/k projection tile mo for token block nb, with RoPE."""
                tok = slice(nb * QB, (nb + 1) * QB)
                ps = half_pool.tile([128, QB], F32, tag="half", name="ps_qk")
                for k in range(KD):
                    nc.tensor.matmul(
                        ps[:],
                        lhsT=wqk_sb[k][:, mo * 128:(mo + 1) * 128],
                        rhs=xts[k][:],
                        start=(k == 0), stop=(k == KD - 1),
                    )
                # RoPE: qk = ps*cos + pairswap(ps)*sig*sin
                swp = rope_pool.tile([128, QB], F32, tag="swp", name="swp")
                nc.vector.stream_shuffle(swp[:], ps[:], SWAP_MASK)
                cq = rope_pool.tile([128, QB], BF16, tag="cq", name="cq")
                nc.vector.scalar_tensor_tensor(
                    cq[:], ps[:], 1.0, ropeC_sb[:, tok],
                    op0=ALU.mult, op1=ALU.mult,
                )
                tm = rope_pool.tile([128, QB], BF16, tag="tm", name="tm")
                nc.vector.scalar_tensor_tensor(
                    tm[:], swp[:], sig_sb[:, 0:1], sinT_sb[:, tok],
                    op0=ALU.mult, op1=ALU.mult,
                )
                nc.gpsimd.tensor_tensor(
                    qk_sb[mo][:, tok], cq[:], tm[:], op=ALU.add)

            def s1_v(nb, mt, xts):
                """v projection for token tile nb*4+mt (token-major + ones)."""
                pv = half_pool.tile([128, QB], F32, tag="half", name="ps_v")
                xsl = slice(mt * 128, (mt + 1) * 128)
                for k in range(KD):
                    nc.tensor.matmul(
                        pv[:],
                        lhsT=xts[k][:, xsl],
                        rhs=wv_sb[k][:],
                        start=(k == 0), stop=(k == KD - 1),
                    )
                vt = v_sb[nb * 4 + mt]
                vdst = vt[:].rearrange("p (h c) -> p h c", h=HL)[:, :, 0:HD]
                nc.vector.tensor_copy(vdst, pv[:].rearrange("p (h c) -> p h c", h=HL))
                ones_ap = vt[:].rearrange("p (h c) -> p h c", h=HL)[:, :, HD]
                nc.vector.memset(ones_ap, 1.0)

            fillers = []       # slow queue: spread over the phase
            fast_fillers = []  # fast queue: one per pair until drained
            quota = {"acc": 0.0, "rate": 1.0}

            def pop_filler():
                if fast_fillers:
                    fast_fillers.pop(0)()
                    return
                quota["acc"] += quota["rate"]
                while fillers and quota["acc"] >= 1.0:
                    quota["acc"] -= 1.0
                    fillers.pop(0)()

            def s2_head(h, qb):
                """Causal attention for head h, query block qb."""
                hp, parity = h // 2, h % 2
                rbase = 64 * parity
                qt = qk_sb[hp]
                kt = qk_sb[4 + hp]
                qsl0 = qb * QB
                o_ps = o_ps_pool.tile([128, 4 * VW], F32, tag="ops", name="o_ps")
                njp = 2 * qb + 2

                def issue_st(jp):
                    st = st_pool.tile([128, 1024], F32, tag="st", name="st")
                    cc0 = max(2 * jp - 4 * qb, 0) * TT
                    for half in (0, 1):
                        j = 2 * jp + half
                        nc.tensor.matmul(
                            st[:, half * QB + cc0:(half + 1) * QB],
                            lhsT=kt[rbase:rbase + HD, j * TT:(j + 1) * TT],
                            rhs=qt[rbase:rbase + HD, qsl0 + cc0:qsl0 + QB],
                            start=True, stop=True,
                        )
                    return st, cc0

                st_cur = issue_st(0)
                for jp in range(njp):
                    st, cc0 = st_cur
                    pt = pt_pool.tile([128, 1024], BF16, tag="pt", name="pt")
                    nc.scalar.activation(pt[:, cc0:1024], st[:, cc0:1024],
                                         AF.Exp, scale=1.0 / math.sqrt(HD))
                    st_cur = issue_st(jp + 1) if jp + 1 < njp else None
                    pop_filler()
                    for half in (0, 1):
                        j = 2 * jp + half
                        oi = j - 4 * qb
                        if oi >= 0:
                            dsl = slice(half * QB + oi * TT,
                                        half * QB + (oi + 1) * TT)
                            nc.vector.tensor_tensor(
                                pt[:, dsl], pt[:, dsl], trimask_sb[:],
                                op=ALU.mult)
                    for half in (0, 1):
                        j = 2 * jp + half
                        oi = j - 4 * qb
                        for qc in range(max(oi, 0), 4):
                            nc.tensor.matmul(
                                o_ps[:, qc * VW:(qc + 1) * VW],
                                lhsT=pt[:, half * QB + qc * TT:
                                        half * QB + (qc + 1) * TT],
                                rhs=v_sb[j][:, VW * h:VW * h + VW],
                                start=(j == 0 and qc == 0),
                                stop=(j == 4 * qb + qc),
                            )
                # normalize: o / denom, written into o_pair tiles (bf16)
                rec = rec_pool.tile([128, 4], F32, tag="rec", name="rec")
                dens = o_ps[:].rearrange("p (q c) -> p q c", q=4)[:, :, HD]
                nc.vector.reciprocal(rec[:], dens)
                return o_ps, rec

            opair_tiles = {}

            def s2_norm(h, qb, o_ps, rec):
                hp, parity = h // 2, h % 2
                key = (hp, qb)
                if key not in opair_tiles:
                    if qb < 3:
                        opair_tiles[key] = opair_pool.tile(
                            [128, QB], BF16, tag="opair", name="opair")
                    else:
                        opair_tiles[key] = opair_pool.tile(
                            [128, QB], F32, tag="opair3", name="opair3")
                opt = opair_tiles[key]
                for qc in range(4):
                    nc.vector.tensor_scalar(
                        opt[:, qc * TT + 64 * parity:qc * TT + 64 * parity + HD],
                        o_ps[:, qc * VW:qc * VW + HD],
                        rec[:, qc:qc + 1], None, op0=ALU.mult)

            def s2_transpose(hp, qb):
                """Transpose the finished o_pair row into oT_sb[hp].

                qb<3 uses the xbar DMA transpose; its consumers run a phase
                later, far beyond the transpose's completion.  qb==3 feeds
                the epilogue within ~1us, and the xbar transpose's
                completion semaphore can fire before all 16 engine chunks
                land, so the epilogue uses PE transposes with exact
                semaphore tracking instead.
                """
                opt = opair_tiles.pop((hp, qb))
                if qb < 3:
                    dst = oT_sb[hp][:, qb * QB:(qb + 1) * QB].rearrange(
                        "p (b q) -> p b q", b=4)
                    nc.sync.dma_start(dst, opt[:], transpose=True)
                    return
                for qc in range(4):
                    qtile = 4 * qb + qc
                    tp_t = half_pool.tile([128, QB], F32, tag="half",
                                          name="tp")
                    nc.tensor.transpose(
                        tp_t[:, 0:128], opt[:, qc * TT:(qc + 1) * TT],
                        identf_sb[:])
                    nc.vector.tensor_copy(
                        oT_sb[hp][:, qtile * TT:(qtile + 1) * TT],
                        tp_t[:, 0:128])

            ot_tiles = {}

            def s3_group(mtt, ib, epi=False):
                tsl = slice(mtt * 128, (mtt + 1) * 128)
                if epi:
                    pot = st_pool.tile([128, 1024], F32, tag="st", name="po_e")
                    po = pot[:, 0:QB]
                else:
                    po = half_pool.tile([128, QB], F32, tag="half", name="po")
                for hp in range(4):
                    nc.tensor.matmul(
                        po[:],
                        lhsT=oT_sb[hp][:, tsl],
                        rhs=wout_sb[hp][:, ib * 512:(ib + 1) * 512],
                        start=(hp == 0), stop=(hp == 3),
                    )
                if mtt not in ot_tiles:
                    ot_tiles[mtt] = out_pool.tile([128, D], F32, tag="ot",
                                                  name="ot")
                ot = ot_tiles[mtt]
                nc.vector.tensor_copy(ot[:, ib * 512:(ib + 1) * 512], po[:])
                if ib == 1:
                    del ot_tiles[mtt]
                    nc.sync.dma_start(outp[tsl, :], ot[:])

            # ---------------- schedule ----------------
            # stage1 block 0 fully first (prologue)
            for mo in range(8):
                s1_qk(0, mo, xts_cur)
            for mt in range(4):
                s1_v(0, mt, xts_cur)

            def F(fn, *args):
                return lambda: fn(*args)

            s3_todo = []   # deferred stage-3 groups from previous qb
            xts_last = None
            for qb in range(nQB):
                xts_next = load_x(qb + 1) if qb + 1 < nQB else None
                if qb == 2:
                    xts_last = xts_next
                for hp in range(4):
                    for parity in (0, 1):
                        h = 2 * hp + parity
                        o_ps, rec = s2_head(h, qb)
                        s2_norm(h, qb, o_ps, rec)
                    s2_transpose(hp, qb)
                    if xts_next is not None:
                        s1_qk(qb + 1, 2 * hp, xts_next)
                        s1_qk(qb + 1, 2 * hp + 1, xts_next)
                        if hp % 2 == 1:
                            s1_v(qb + 1, hp - 1, xts_next)
                            s1_v(qb + 1, hp, xts_next)
                    n_s3 = {0: 0, 1: 1, 2: 2, 3: 5}[qb]
                    for _ in range(n_s3):
                        if s3_todo:
                            s3_group(*s3_todo.pop(0))
                for mtt in range(4 * qb, 4 * qb + 4):
                    for ib in range(2):
                        s3_todo.append((mtt, ib))
            epi = True
            while s3_todo:
                mtt, ib = s3_todo.pop(0)
                s3_group(mtt, ib, epi=epi)
                epi = not epi
            if debug:
                for i in range(8):
                    nc.sync.dma_start(qk_dbg[i * 128:(i + 1) * 128, :], qk_sb[i][:])
                for i in range(16):
                    nc.sync.dma_start(v_dbg[i * 128:(i + 1) * 128, :], v_sb[i][:])
                for i in range(4):
                    nc.sync.dma_start(oT_dbg[i * 128:(i + 1) * 128, :], oT_sb[i][:])

    nc.compile()
    return nc


# ---------------------------------------------------------------------------
# host side
# ---------------------------------------------------------------------------

_cache = {}


def _get_nc(S):
    if S not in _cache:
        _cache[S] = build_nc(S)
    return _cache[S]


def _rope_perm():
    """Row permutation within a 64-dim head: r = 2i+p  <-  i + 32p."""
    perm = np.empty(64, dtype=np.int64)
    for i in range(32):
        for p in (0, 1):
            perm[2 * i + p] = i + 32 * p
    return perm


def _shard_weights(w_qkv, w_out, g):
    """Per-head-group weight shards in device layouts (bf16)."""
    w_qkv = np.asarray(w_qkv, dtype=np.float32)
    w_out = np.asarray(w_out, dtype=np.float32)
    perm = _rope_perm()
    rows = []
    for part in range(2):           # 0: q, 1: k
        base = part * D
        for hl in range(HL):
            h_glob = g * HL + hl
            blk = w_qkv[base + h_glob * HD: base + (h_glob + 1) * HD]
            rows.append(blk[perm])
    wqk = np.concatenate(rows, axis=0)                 # [1024, 1024]
    wqkT = np.ascontiguousarray(wqk.T).astype(BF)      # [D, 1024]

    r = slice(2 * D + g * 512, 2 * D + (g + 1) * 512)
    wv = w_qkv[r]                                      # [512, 1024]
    wvT = np.ascontiguousarray(wv.T).astype(BF)        # [D, 512]

    woutT = np.ascontiguousarray(
        w_out.T[g * 512:(g + 1) * 512]).astype(BF)     # [512, 1024]
    return wqkT, wvT, woutT


def kernel(x, w_qkv, w_out):
    x = np.asarray(x, dtype=np.float32)
    B, S, _D = x.shape
    assert _D == D
    nc = _get_nc(S)

    shards = [_shard_weights(w_qkv, w_out, g) for g in range(2)]
    in_maps = []
    for core in range(8):
        b, g = core // 2, core % 2
        wqkT, wvT, woutT = shards[g]
        in_maps.append({
            "xT": np.ascontiguousarray(x[b].T).astype(BF),
            "wqkT": wqkT,
            "wvT": wvT,
            "woutT": woutT,
        })
    res = run_bass_kernel_spmd(nc, in_maps, list(range(8)))
    out = np.empty((B, S, D), dtype=np.float32)
    for b in range(B):
        out[b] = res.results[2 * b]["outp"] + res.results[2 * b + 1]["outp"]
    return out


# revision 5
# speedup vs baseline: 1.0048x; 1.0012x over previous
"""Multi-head causal attention (B=4, S=2048, D=1024, H=16, RoPE) on 8 TRN2 cores.

v2: bf16 compute, token-major PV with ones-column denominators, stream-shuffle
RoPE (pair-interleaved head dims), wide softmax exps, cross-stage interleaving.

Sharding: core = (batch b, head-group g of 8 heads).  Each core computes
qkv projection for its (b, g), RoPE, causal attention, and a partial
out-projection (contraction over its 512 head-dims).  Host sums the two
partials per batch.

Device layouts (per core):
  qk_sb[mo] [128, S]  feature-major bf16; mo 0..3 = q head pairs, 4..7 = k.
                      rows: head-local u*64 + 2*i + p  <->  orig dim i + 32p
                      (pair-interleaved so RoPE rotate-half is an adjacent-pair
                      partition swap, done by stream_shuffle within quadrants)
  v_sb[j]   [128, 520] token-major bf16, 65 cols/head: 64 dims + ones column
                      that makes the P@V matmul accumulate softmax denominators
  st        [t, q]    scores transposed (k-major) so they feed PV as lhsT
  o_ps      [q, 65]   per-(head,q-chunk) attention out + denominator column
  oT_sb[hp] [128, S]  transposed normalized outputs (PE transpose), out-proj lhsT
"""

import math

import numpy as np
import ml_dtypes

import concourse.bass as bass
import concourse.bacc as bacc
import concourse.mybir as mybir
from concourse import tile
from concourse.bass_utils import run_bass_kernel_spmd

AF = mybir.ActivationFunctionType
ALU = mybir.AluOpType
F32 = mybir.dt.float32
BF16 = mybir.dt.bfloat16

N_HEADS = 16
THETA = 10000.0
D = 1024
HD = 64
HL = 8          # heads per core
VW = HD + 1     # v cols per head (64 dims + ones)
QB = 512        # query block / stage-1 token block
TT = 128        # key/value tile
BF = ml_dtypes.bfloat16

SWAP_MASK = []
for _i in range(16):
    SWAP_MASK += [2 * _i + 1, 2 * _i]


def _host_constants(S):
    """RoPE tables (pair-interleaved rows), sign vector, triangle mask."""
    half = HD // 2
    inv = 1.0 / (THETA ** (np.arange(half, dtype=np.float64) / half))
    t = np.arange(S, dtype=np.float64)
    ang = inv[:, None] * t[None, :]                      # [32, S]
    cos = np.cos(ang)
    sin = np.sin(ang)
    # rows: u*64 + 2i + p  -> freq i (independent of u, p)
    cosr = np.repeat(cos, 2, axis=0)                     # [64, S]
    sinr = np.repeat(sin, 2, axis=0)
    ropeC = np.tile(cosr, (2, 1)).astype(BF)             # [128, S]
    sinT = np.tile(sinr, (2, 1)).astype(BF)
    sig = np.tile(np.array([-1.0, 1.0]), 64).astype(np.float32)[:, None]  # [128,1]
    p = np.arange(128)[:, None]
    c = np.arange(128)[None, :]
    trimask = (c >= p).astype(BF)                        # [128, 128]
    ident = np.eye(128, dtype=BF)
    return ropeC, sinT, sig, trimask, ident


def build_nc(S=2048, debug=False):
    nc = bacc.Bacc("TRN2", target_bir_lowering=False, debug=False)

    xT = nc.dram_tensor("xT", [D, S], BF16, kind="ExternalInput").ap()
    wqkT = nc.dram_tensor("wqkT", [D, 2 * HL * HD], BF16, kind="ExternalInput").ap()
    wvT = nc.dram_tensor("wvT", [D, HL * HD], BF16, kind="ExternalInput").ap()
    woutT = nc.dram_tensor("woutT", [HL * HD, D], BF16, kind="ExternalInput").ap()
    outp = nc.dram_tensor("outp", [S, D], F32, kind="ExternalOutput").ap()
    if debug:
        qk_dbg = nc.dram_tensor("qk_dbg", [8 * 128, S], BF16, kind="ExternalOutput").ap()
        v_dbg = nc.dram_tensor("v_dbg", [16 * 128, HL * VW], BF16, kind="ExternalOutput").ap()
        oT_dbg = nc.dram_tensor("oT_dbg", [4 * 128, S], BF16, kind="ExternalOutput").ap()

    ropeC_np, sinT_np, sig_np, trimask_np, ident_np = _host_constants(S)
    tabs_np = np.concatenate([ropeC_np, sinT_np, trimask_np, ident_np], axis=1)
    tabs_d = nc.inline_tensor(tabs_np, "tabs").ap()
    sig_d = nc.inline_tensor(sig_np, "sig").ap()
    identf_d = nc.inline_tensor(ident_np.astype(np.float32), "identf").ap()

    KD = D // 128        # 8 contraction tiles
    nQB = S // QB        # 4
    nMT = S // TT        # 16

    with tile.TileContext(nc) as tc:
        with (
            tc.tile_pool(name="qk", bufs=1) as qk_pool,
            tc.tile_pool(name="vres", bufs=1) as v_pool,
            tc.tile_pool(name="osb", bufs=1) as o_pool,
            tc.tile_pool(name="wqk", bufs=1) as wqk_pool,
            tc.tile_pool(name="wv", bufs=1) as wv_pool,
            tc.tile_pool(name="wout", bufs=1) as wout_pool,
            tc.tile_pool(name="tabs", bufs=1) as tab_pool,
            tc.tile_pool(name="xs", bufs=2) as x_pool,
            tc.tile_pool(name="rope", bufs=3) as rope_pool,
            tc.tile_pool(name="pt", bufs=3) as pt_pool,
            tc.tile_pool(name="opair", bufs=3) as opair_pool,
            tc.tile_pool(name="rec", bufs=4) as rec_pool,
            tc.tile_pool(name="outs", bufs=4) as out_pool,
            tc.tile_pool(name="ps_half", bufs=2, space="PSUM") as half_pool,
            tc.tile_pool(name="ps_st", bufs=2, space="PSUM") as st_pool,
            tc.tile_pool(name="ps_o", bufs=2, space="PSUM") as o_ps_pool,
        ):
            qk_sb = [qk_pool.tile([128, S], BF16, tag=f"qk{i}", name=f"qk{i}")
                     for i in range(8)]
            v_sb = [v_pool.tile([128, HL * VW], BF16, tag=f"v{i}", name=f"v{i}")
                    for i in range(nMT)]
            oT_sb = [o_pool.tile([128, S], BF16, tag=f"oT{i}", name=f"oT{i}")
                     for i in range(4)]
            wqk_big = wqk_pool.tile([128, KD * 128 * 8], BF16, tag="wqk",
                                    name="wqk_big")
            wv_big = wv_pool.tile([128, KD * HL * HD], BF16, tag="wv",
                                  name="wv_big")
            wout_big = wout_pool.tile([128, 4 * D], BF16, tag="wo",
                                      name="wout_big")
            wqk_sb = [wqk_big[:, k * 1024:(k + 1) * 1024] for k in range(KD)]
            wv_sb = [wv_big[:, k * 512:(k + 1) * 512] for k in range(KD)]
            wout_sb = [wout_big[:, i * D:(i + 1) * D] for i in range(4)]
            tabs_sb = tab_pool.tile([128, 2 * S + 256], BF16, tag="tabs",
                                    name="tabs")
            ropeC_sb = tabs_sb[:, 0:S]
            sinT_sb = tabs_sb[:, S:2 * S]
            trimask_sb = tabs_sb[:, 2 * S:2 * S + 128]
            ident_sb = tabs_sb[:, 2 * S + 128:2 * S + 256]
            sig_sb = tab_pool.tile([128, 1], F32, tag="sig", name="sig")
            identf_sb = tab_pool.tile([128, 128], F32, tag="identf",
                                      name="identf")

            # ---------------- prologue DMAs (one big DMA per tensor) -------
            def load_x(nb):
                tk = slice(nb * QB, (nb + 1) * QB)
                xt = x_pool.tile([128, KD * QB], BF16, tag="xt", name="xt")
                nc.sync.dma_start(
                    xt[:].rearrange("p (k c) -> p k c", k=KD),
                    xT[:, tk].rearrange("(k p) c -> p k c", p=128))
                return [xt[:, k * QB:(k + 1) * QB] for k in range(KD)]

            xt0 = x_pool.tile([128, KD * QB], BF16, tag="xt", name="xt")
            for lo in range(0, KD, 2):
                nc.sync.dma_start(
                    wqk_big[:, lo * 1024:(lo + 2) * 1024].rearrange(
                        "p (k c) -> p k c", k=2),
                    wqkT[lo * 128:(lo + 2) * 128, :].rearrange(
                        "(k p) c -> p k c", p=128))
                nc.sync.dma_start(
                    xt0[:, lo * QB:(lo + 2) * QB].rearrange(
                        "p (k c) -> p k c", k=2),
                    xT[lo * 128:(lo + 2) * 128, 0:QB].rearrange(
                        "(k p) c -> p k c", p=128))
            xts0 = [xt0[:, k * QB:(k + 1) * QB] for k in range(KD)]
            nc.sync.dma_start(tabs_sb[:], tabs_d[:])
            nc.sync.dma_start(sig_sb[:], sig_d[:])
            nc.sync.dma_start(identf_sb[:], identf_d[:])
            nc.sync.dma_start(
                wv_big[:].rearrange("p (k c) -> p k c", k=KD),
                wvT.rearrange("(k p) c -> p k c", p=128))
            nc.sync.dma_start(
                wout_big[:].rearrange("p (k c) -> p k c", k=4),
                woutT.rearrange("(k p) c -> p k c", p=128))

            xts_cur = xts0

            # ---------------- stage pieces ----------------
            st_half = {"tile": None}

            def deep_ps(mo):
                """Prologue-only: 6-deep PSUM ring borrowing idle st banks."""
                sel = mo % 3
                if sel < 2:
                    if sel == 0:
                        st_half["tile"] = st_pool.tile(
                            [128, 1024], F32, tag="st", name="ps_qk_st")
                        return st_half["tile"][:, 0:QB]
                    return st_half["tile"][:, QB:2 * QB]
                return half_pool.tile([128, QB], F32, tag="half", name="ps_qk")

            def s1_qk(nb, mo, xts):
                """q/k projection tile mo for token block nb, with RoPE."""
                tok = slice(nb * QB, (nb + 1) * QB)
                ps = half_pool.tile([128, QB], F32, tag="half", name="ps_qk")
                for k in range(KD):
                    nc.tensor.matmul(
                        ps[:],
                        lhsT=wqk_sb[k][:, mo * 128:(mo + 1) * 128],
                        rhs=xts[k][:],
                        start=(k == 0), stop=(k == KD - 1),
                    )
                # RoPE: qk = ps*cos + pairswap(ps)*sig*sin
                swp = rope_pool.tile([128, QB], F32, tag="swp", name="swp")
                nc.vector.stream_shuffle(swp[:], ps[:], SWAP_MASK)
                cq = rope_pool.tile([128, QB], BF16, tag="cq", name="cq")
                nc.vector.scalar_tensor_tensor(
                    cq[:], ps[:], 1.0, ropeC_sb[:, tok],
                    op0=ALU.mult, op1=ALU.mult,
                )
                tm = rope_pool.tile([128, QB], BF16, tag="tm", name="tm")
                nc.vector.scalar_tensor_tensor(
                    tm[:], swp[:], sig_sb[:, 0:1], sinT_sb[:, tok],
                    op0=ALU.mult, op1=ALU.mult,
                )
                nc.gpsimd.tensor_tensor(
                    qk_sb[mo][:, tok], cq[:], tm[:], op=ALU.add)

            def s1_v(nb, mt, xts):
                """v projection for token tile nb*4+mt (token-major + ones)."""
                pv = half_pool.tile([128, QB], F32, tag="half", name="ps_v")
                xsl = slice(mt * 128, (mt + 1) * 128)
                for k in range(KD):
                    nc.tensor.matmul(
                        pv[:],
                        lhsT=xts[k][:, xsl],
                        rhs=wv_sb[k][:],
                        start=(k == 0), stop=(k == KD - 1),
                    )
                vt = v_sb[nb * 4 + mt]
                vdst = vt[:].rearrange("p (h c) -> p h c", h=HL)[:, :, 0:HD]
                nc.vector.tensor_copy(vdst, pv[:].rearrange("p (h c) -> p h c", h=HL))
                ones_ap = vt[:].rearrange("p (h c) -> p h c", h=HL)[:, :, HD]
                nc.vector.memset(ones_ap, 1.0)

            fillers = []       # slow queue: spread over the phase
            fast_fillers = []  # fast queue: one per pair until drained
            quota = {"acc": 0.0, "rate": 1.0}

            def pop_filler():
                if fast_fillers:
                    fast_fillers.pop(0)()
                    return
                quota["acc"] += quota["rate"]
                while fillers and quota["acc"] >= 1.0:
                    quota["acc"] -= 1.0
                    fillers.pop(0)()

            def s2_head(h, qb):
                """Causal attention for head h, query block qb."""
                hp, parity = h // 2, h % 2
                rbase = 64 * parity
                qt = qk_sb[hp]
                kt = qk_sb[4 + hp]
                qsl0 = qb * QB
                o_ps = o_ps_pool.tile([128, 4 * VW], F32, tag="ops", name="o_ps")
                njp = 2 * qb + 2

                def issue_st(jp):
                    st = st_pool.tile([128, 1024], F32, tag="st", name="st")
                    cc0 = max(2 * jp - 4 * qb, 0) * TT
                    for half in (0, 1):
                        j = 2 * jp + half
                        nc.tensor.matmul(
                            st[:, half * QB + cc0:(half + 1) * QB],
                            lhsT=kt[rbase:rbase + HD, j * TT:(j + 1) * TT],
                            rhs=qt[rbase:rbase + HD, qsl0 + cc0:qsl0 + QB],
                            start=True, stop=True,
                        )
                    return st, cc0

                st_cur = issue_st(0)
                for jp in range(njp):
                    st, cc0 = st_cur
                    pt = pt_pool.tile([128, 1024], BF16, tag="pt", name="pt")
                    nc.scalar.activation(pt[:, cc0:1024], st[:, cc0:1024],
                                         AF.Exp, scale=1.0 / math.sqrt(HD))
                    st_cur = issue_st(jp + 1) if jp + 1 < njp else None
                    pop_filler()
                    for half in (0, 1):
                        j = 2 * jp + half
                        oi = j - 4 * qb
                        if oi >= 0:
                            dsl = slice(half * QB + oi * TT,
                                        half * QB + (oi + 1) * TT)
                            nc.vector.tensor_tensor(
                                pt[:, dsl], pt[:, dsl], trimask_sb[:],
                                op=ALU.mult)
                    for half in (0, 1):
                        j = 2 * jp + half
                        oi = j - 4 * qb
                        for qc in range(max(oi, 0), 4):
                            nc.tensor.matmul(
                                o_ps[:, qc * VW:(qc + 1) * VW],
                                lhsT=pt[:, half * QB + qc * TT:
                                        half * QB + (qc + 1) * TT],
                                rhs=v_sb[j][:, VW * h:VW * h + VW],
                                start=(j == 0 and qc == 0),
                                stop=(j == 4 * qb + qc),
                            )
                # normalize: o / denom, written into o_pair tiles (bf16)
                rec = rec_pool.tile([128, 4], F32, tag="rec", name="rec")
                dens = o_ps[:].rearrange("p (q c) -> p q c", q=4)[:, :, HD]
                nc.vector.reciprocal(rec[:], dens)
                return o_ps, rec

            opair_tiles = {}

            def s2_norm(h, qb, o_ps, rec):
                hp, parity = h // 2, h % 2
                key = (hp, qb)
                if key not in opair_tiles:
                    if qb < 3:
                        opair_tiles[key] = opair_pool.tile(
                            [128, QB], BF16, tag="opair", name="opair")
                    else:
                        opair_tiles[key] = opair_pool.tile(
                            [128, QB], F32, tag="opair3", name="opair3")
                opt = opair_tiles[key]
                for qc in range(4):
                    nc.vector.tensor_scalar(
                        opt[:, qc * TT + 64 * parity:qc * TT + 64 * parity + HD],
                        o_ps[:, qc * VW:qc * VW + HD],
                        rec[:, qc:qc + 1], None, op0=ALU.mult)

            def s2_transpose(hp, qb):
                """Transpose the finished o_pair row into oT_sb[hp].

                qb<3 uses the xbar DMA transpose; its consumers run a phase
                later, far beyond the transpose's completion.  qb==3 feeds
                the epilogue within ~1us, and the xbar transpose's
                completion semaphore can fire before all 16 engine chunks
                land, so the epilogue uses PE transposes with exact
                semaphore tracking instead.
                """
                opt = opair_tiles.pop((hp, qb))
                if qb < 3:
                    dst = oT_sb[hp][:, qb * QB:(qb + 1) * QB].rearrange(
                        "p (b q) -> p b q", b=4)
                    nc.sync.dma_start(dst, opt[:], transpose=True)
                    return
                for qc in range(4):
                    qtile = 4 * qb + qc
                    tp_t = half_pool.tile([128, QB], F32, tag="half",
                                          name="tp")
                    nc.tensor.transpose(
                        tp_t[:, 0:128], opt[:, qc * TT:(qc + 1) * TT],
                        identf_sb[:])
                    nc.vector.tensor_copy(
                        oT_sb[hp][:, qtile * TT:(qtile + 1) * TT],
                        tp_t[:, 0:128])

            def s3_group(mtt, ib, epi=False):
                tsl = slice(mtt * 128, (mtt + 1) * 128)
                if epi:
                    pot = st_pool.tile([128, 1024], F32, tag="st", name="po_e")
                    po = pot[:, 0:QB]
                else:
                    po = half_pool.tile([128, QB], F32, tag="half", name="po")
                for hp in range(4):
                    nc.tensor.matmul(
                        po[:],
                        lhsT=oT_sb[hp][:, tsl],
                        rhs=wout_sb[hp][:, ib * 512:(ib + 1) * 512],
                        start=(hp == 0), stop=(hp == 3),
                    )
                ot = out_pool.tile([128, QB], F32, tag="ot", name="ot")
                # epilogue halves alternate Act/DVE so the final copy+store
                # chains of the last token tiles pipeline two-wide
                if epi and ib == 0:
                    nc.scalar.copy(ot[:], po[:])
                else:
                    nc.vector.tensor_copy(ot[:], po[:])
                nc.sync.dma_start(outp[tsl, ib * 512:(ib + 1) * 512], ot[:])

            # ---------------- schedule ----------------
            # stage1 block 0 fully first (prologue)
            for mo in range(8):
                s1_qk(0, mo, xts_cur)
            for mt in range(4):
                s1_v(0, mt, xts_cur)

            def F(fn, *args):
                return lambda: fn(*args)

            s3_todo = []   # deferred stage-3 groups from previous qb
            xts_last = None
            for qb in range(nQB):
                xts_next = load_x(qb + 1) if qb + 1 < nQB else None
                if qb == 2:
                    xts_last = xts_next
                for hp in range(4):
                    for parity in (0, 1):
                        h = 2 * hp + parity
                        o_ps, rec = s2_head(h, qb)
                        s2_norm(h, qb, o_ps, rec)
                    s2_transpose(hp, qb)
                    if xts_next is not None:
                        s1_qk(qb + 1, 2 * hp, xts_next)
                        s1_qk(qb + 1, 2 * hp + 1, xts_next)
                        if hp % 2 == 1:
                            s1_v(qb + 1, hp - 1, xts_next)
                            s1_v(qb + 1, hp, xts_next)
                    n_s3 = {0: 0, 1: 1, 2: 2, 3: 5}[qb]
                    for _ in range(n_s3):
                        if s3_todo:
                            s3_group(*s3_todo.pop(0))
                for mtt in range(4 * qb, 4 * qb + 4):
                    for ib in range(2):
                        s3_todo.append((mtt, ib))
            epi = True
            while s3_todo:
                mtt, ib = s3_todo.pop(0)
                s3_group(mtt, ib, epi=epi)
                epi = not epi
            if debug:
                for i in range(8):
                    nc.sync.dma_start(qk_dbg[i * 128:(i + 1) * 128, :], qk_sb[i][:])
                for i in range(16):
                    nc.sync.dma_start(v_dbg[i * 128:(i + 1) * 128, :], v_sb[i][:])
                for i in range(4):
                    nc.sync.dma_start(oT_dbg[i * 128:(i + 1) * 128, :], oT_sb[i][:])

    nc.compile()
    return nc


# ---------------------------------------------------------------------------
# host side
# ---------------------------------------------------------------------------

_cache = {}


def _get_nc(S):
    if S not in _cache:
        _cache[S] = build_nc(S)
    return _cache[S]


def _rope_perm():
    """Row permutation within a 64-dim head: r = 2i+p  <-  i + 32p."""
    perm = np.empty(64, dtype=np.int64)
    for i in range(32):
        for p in (0, 1):
            perm[2 * i + p] = i + 32 * p
    return perm


def _shard_weights(w_qkv, w_out, g):
    """Per-head-group weight shards in device layouts (bf16)."""
    w_qkv = np.asarray(w_qkv, dtype=np.float32)
    w_out = np.asarray(w_out, dtype=np.float32)
    perm = _rope_perm()
    rows = []
    for part in range(2):           # 0: q, 1: k
        base = part * D
        for hl in range(HL):
            h_glob = g * HL + hl
            blk = w_qkv[base + h_glob * HD: base + (h_glob + 1) * HD]
            rows.append(blk[perm])
    wqk = np.concatenate(rows, axis=0)                 # [1024, 1024]
    wqkT = np.ascontiguousarray(wqk.T).astype(BF)      # [D, 1024]

    r = slice(2 * D + g * 512, 2 * D + (g + 1) * 512)
    wv = w_qkv[r]                                      # [512, 1024]
    wvT = np.ascontiguousarray(wv.T).astype(BF)        # [D, 512]

    woutT = np.ascontiguousarray(
        w_out.T[g * 512:(g + 1) * 512]).astype(BF)     # [512, 1024]
    return wqkT, wvT, woutT


def kernel(x, w_qkv, w_out):
    x = np.asarray(x, dtype=np.float32)
    B, S, _D = x.shape
    assert _D == D
    nc = _get_nc(S)

    shards = [_shard_weights(w_qkv, w_out, g) for g in range(2)]
    in_maps = []
    for core in range(8):
        b, g = core // 2, core % 2
        wqkT, wvT, woutT = shards[g]
        in_maps.append({
            "xT": np.ascontiguousarray(x[b].T).astype(BF),
            "wqkT": wqkT,
            "wvT": wvT,
            "woutT": woutT,
        })
    res = run_bass_kernel_spmd(nc, in_maps, list(range(8)))
    out = np.empty((B, S, D), dtype=np.float32)
    for b in range(B):
        out[b] = res.results[2 * b]["outp"] + res.results[2 * b + 1]["outp"]
    return out


# revision 6
# speedup vs baseline: 1.0074x; 1.0026x over previous
"""Multi-head causal attention (B=4, S=2048, D=1024, H=16, RoPE) on 8 TRN2 cores.

v2: bf16 compute, token-major PV with ones-column denominators, stream-shuffle
RoPE (pair-interleaved head dims), wide softmax exps, cross-stage interleaving.

Sharding: core = (batch b, head-group g of 8 heads).  Each core computes
qkv projection for its (b, g), RoPE, causal attention, and a partial
out-projection (contraction over its 512 head-dims).  Host sums the two
partials per batch.

Device layouts (per core):
  qk_sb[mo] [128, S]  feature-major bf16; mo 0..3 = q head pairs, 4..7 = k.
                      rows: head-local u*64 + 2*i + p  <->  orig dim i + 32p
                      (pair-interleaved so RoPE rotate-half is an adjacent-pair
                      partition swap, done by stream_shuffle within quadrants)
  v_sb[j]   [128, 520] token-major bf16, 65 cols/head: 64 dims + ones column
                      that makes the P@V matmul accumulate softmax denominators
  st        [t, q]    scores transposed (k-major) so they feed PV as lhsT
  o_ps      [q, 65]   per-(head,q-chunk) attention out + denominator column
  oT_sb[hp] [128, S]  transposed normalized outputs (PE transpose), out-proj lhsT
"""

import math

import numpy as np
import ml_dtypes

import concourse.bass as bass
import concourse.bacc as bacc
import concourse.mybir as mybir
from concourse import tile
from concourse.bass_utils import run_bass_kernel_spmd

AF = mybir.ActivationFunctionType
ALU = mybir.AluOpType
F32 = mybir.dt.float32
BF16 = mybir.dt.bfloat16

N_HEADS = 16
THETA = 10000.0
D = 1024
HD = 64
HL = 8          # heads per core
VW = HD + 1     # v cols per head (64 dims + ones)
QB = 512        # query block / stage-1 token block
TT = 128        # key/value tile
BF = ml_dtypes.bfloat16

SWAP_MASK = []
for _i in range(16):
    SWAP_MASK += [2 * _i + 1, 2 * _i]


def _host_constants(S):
    """RoPE tables (pair-interleaved rows), sign vector, triangle mask."""
    half = HD // 2
    inv = 1.0 / (THETA ** (np.arange(half, dtype=np.float64) / half))
    t = np.arange(S, dtype=np.float64)
    ang = inv[:, None] * t[None, :]                      # [32, S]
    cos = np.cos(ang)
    sin = np.sin(ang)
    # rows: u*64 + 2i + p  -> freq i (independent of u, p)
    cosr = np.repeat(cos, 2, axis=0)                     # [64, S]
    sinr = np.repeat(sin, 2, axis=0)
    ropeC = np.tile(cosr, (2, 1)).astype(BF)             # [128, S]
    sinT = np.tile(sinr, (2, 1)).astype(BF)
    sig = np.tile(np.array([-1.0, 1.0]), 64).astype(np.float32)[:, None]  # [128,1]
    p = np.arange(128)[:, None]
    c = np.arange(128)[None, :]
    trimask = (c >= p).astype(BF)                        # [128, 128]
    ident = np.eye(128, dtype=BF)
    return ropeC, sinT, sig, trimask, ident


def build_nc(S=2048, debug=False):
    nc = bacc.Bacc("TRN2", target_bir_lowering=False, debug=False)

    xT = nc.dram_tensor("xT", [D, S], BF16, kind="ExternalInput").ap()
    wqkT = nc.dram_tensor("wqkT", [D, 2 * HL * HD], BF16, kind="ExternalInput").ap()
    wvT = nc.dram_tensor("wvT", [D, HL * HD], BF16, kind="ExternalInput").ap()
    woutT = nc.dram_tensor("woutT", [HL * HD, D], BF16, kind="ExternalInput").ap()
    outp = nc.dram_tensor("outp", [S, D], F32, kind="ExternalOutput").ap()
    if debug:
        qk_dbg = nc.dram_tensor("qk_dbg", [8 * 128, S], BF16, kind="ExternalOutput").ap()
        v_dbg = nc.dram_tensor("v_dbg", [16 * 128, HL * VW], BF16, kind="ExternalOutput").ap()
        oT_dbg = nc.dram_tensor("oT_dbg", [4 * 128, S], BF16, kind="ExternalOutput").ap()

    ropeC_np, sinT_np, sig_np, trimask_np, ident_np = _host_constants(S)
    tabs_np = np.concatenate([ropeC_np, sinT_np, trimask_np, ident_np], axis=1)
    tabs_d = nc.inline_tensor(tabs_np, "tabs").ap()
    sig_d = nc.inline_tensor(sig_np, "sig").ap()
    identf_d = nc.inline_tensor(ident_np.astype(np.float32), "identf").ap()

    KD = D // 128        # 8 contraction tiles
    nQB = S // QB        # 4
    nMT = S // TT        # 16

    with tile.TileContext(nc) as tc:
        with (
            tc.tile_pool(name="qk", bufs=1) as qk_pool,
            tc.tile_pool(name="vres", bufs=1) as v_pool,
            tc.tile_pool(name="osb", bufs=1) as o_pool,
            tc.tile_pool(name="wqk", bufs=1) as wqk_pool,
            tc.tile_pool(name="wv", bufs=1) as wv_pool,
            tc.tile_pool(name="wout", bufs=1) as wout_pool,
            tc.tile_pool(name="tabs", bufs=1) as tab_pool,
            tc.tile_pool(name="xs", bufs=2) as x_pool,
            tc.tile_pool(name="rope", bufs=3) as rope_pool,
            tc.tile_pool(name="pt", bufs=3) as pt_pool,
            tc.tile_pool(name="opair", bufs=3) as opair_pool,
            tc.tile_pool(name="rec", bufs=4) as rec_pool,
            tc.tile_pool(name="outs", bufs=6) as out_pool,
            tc.tile_pool(name="ps_half", bufs=2, space="PSUM") as half_pool,
            tc.tile_pool(name="ps_st", bufs=2, space="PSUM") as st_pool,
            tc.tile_pool(name="ps_o", bufs=2, space="PSUM") as o_ps_pool,
        ):
            qk_sb = [qk_pool.tile([128, S], BF16, tag=f"qk{i}", name=f"qk{i}")
                     for i in range(8)]
            v_sb = [v_pool.tile([128, HL * VW], BF16, tag=f"v{i}", name=f"v{i}")
                    for i in range(nMT)]
            oT_sb = [o_pool.tile([128, S], BF16, tag=f"oT{i}", name=f"oT{i}")
                     for i in range(4)]
            wqk_big = wqk_pool.tile([128, KD * 128 * 8], BF16, tag="wqk",
                                    name="wqk_big")
            wv_big = wv_pool.tile([128, KD * HL * HD], BF16, tag="wv",
                                  name="wv_big")
            wout_big = wout_pool.tile([128, 4 * D], BF16, tag="wo",
                                      name="wout_big")
            wqk_sb = [wqk_big[:, k * 1024:(k + 1) * 1024] for k in range(KD)]
            wv_sb = [wv_big[:, k * 512:(k + 1) * 512] for k in range(KD)]
            wout_sb = [wout_big[:, i * D:(i + 1) * D] for i in range(4)]
            tabs_sb = tab_pool.tile([128, 2 * S + 256], BF16, tag="tabs",
                                    name="tabs")
            ropeC_sb = tabs_sb[:, 0:S]
            sinT_sb = tabs_sb[:, S:2 * S]
            trimask_sb = tabs_sb[:, 2 * S:2 * S + 128]
            ident_sb = tabs_sb[:, 2 * S + 128:2 * S + 256]
            sig_sb = tab_pool.tile([128, 1], F32, tag="sig", name="sig")
            identf_sb = tab_pool.tile([128, 128], F32, tag="identf",
                                      name="identf")

            # ---------------- prologue DMAs (one big DMA per tensor) -------
            def load_x(nb):
                tk = slice(nb * QB, (nb + 1) * QB)
                xt = x_pool.tile([128, KD * QB], BF16, tag="xt", name="xt")
                nc.sync.dma_start(
                    xt[:].rearrange("p (k c) -> p k c", k=KD),
                    xT[:, tk].rearrange("(k p) c -> p k c", p=128))
                return [xt[:, k * QB:(k + 1) * QB] for k in range(KD)]

            xt0 = x_pool.tile([128, KD * QB], BF16, tag="xt", name="xt")
            for lo in range(0, KD, 2):
                nc.sync.dma_start(
                    wqk_big[:, lo * 1024:(lo + 2) * 1024].rearrange(
                        "p (k c) -> p k c", k=2),
                    wqkT[lo * 128:(lo + 2) * 128, :].rearrange(
                        "(k p) c -> p k c", p=128))
                nc.sync.dma_start(
                    xt0[:, lo * QB:(lo + 2) * QB].rearrange(
                        "p (k c) -> p k c", k=2),
                    xT[lo * 128:(lo + 2) * 128, 0:QB].rearrange(
                        "(k p) c -> p k c", p=128))
            xts0 = [xt0[:, k * QB:(k + 1) * QB] for k in range(KD)]
            nc.sync.dma_start(tabs_sb[:], tabs_d[:])
            nc.sync.dma_start(sig_sb[:], sig_d[:])
            nc.sync.dma_start(identf_sb[:], identf_d[:])
            nc.sync.dma_start(
                wv_big[:].rearrange("p (k c) -> p k c", k=KD),
                wvT.rearrange("(k p) c -> p k c", p=128))
            nc.sync.dma_start(
                wout_big[:].rearrange("p (k c) -> p k c", k=4),
                woutT.rearrange("(k p) c -> p k c", p=128))

            xts_cur = xts0

            # ---------------- stage pieces ----------------
            st_half = {"tile": None}

            def deep_ps(mo):
                """Prologue-only: 6-deep PSUM ring borrowing idle st banks."""
                sel = mo % 3
                if sel < 2:
                    if sel == 0:
                        st_half["tile"] = st_pool.tile(
                            [128, 1024], F32, tag="st", name="ps_qk_st")
                        return st_half["tile"][:, 0:QB]
                    return st_half["tile"][:, QB:2 * QB]
                return half_pool.tile([128, QB], F32, tag="half", name="ps_qk")

            def s1_qk(nb, mo, xts):
                """q/k projection tile mo for token block nb, with RoPE."""
                tok = slice(nb * QB, (nb + 1) * QB)
                ps = half_pool.tile([128, QB], F32, tag="half", name="ps_qk")
                for k in range(KD):
                    nc.tensor.matmul(
                        ps[:],
                        lhsT=wqk_sb[k][:, mo * 128:(mo + 1) * 128],
                        rhs=xts[k][:],
                        start=(k == 0), stop=(k == KD - 1),
                    )
                # RoPE: qk = ps*cos + pairswap(ps)*sig*sin
                swp = rope_pool.tile([128, QB], F32, tag="swp", name="swp")
                nc.vector.stream_shuffle(swp[:], ps[:], SWAP_MASK)
                cq = rope_pool.tile([128, QB], BF16, tag="cq", name="cq")
                nc.vector.scalar_tensor_tensor(
                    cq[:], ps[:], 1.0, ropeC_sb[:, tok],
                    op0=ALU.mult, op1=ALU.mult,
                )
                tm = rope_pool.tile([128, QB], BF16, tag="tm", name="tm")
                nc.vector.scalar_tensor_tensor(
                    tm[:], swp[:], sig_sb[:, 0:1], sinT_sb[:, tok],
                    op0=ALU.mult, op1=ALU.mult,
                )
                nc.gpsimd.tensor_tensor(
                    qk_sb[mo][:, tok], cq[:], tm[:], op=ALU.add)

            def s1_v(nb, mt, xts):
                """v projection for token tile nb*4+mt (token-major + ones)."""
                pv = half_pool.tile([128, QB], F32, tag="half", name="ps_v")
                xsl = slice(mt * 128, (mt + 1) * 128)
                for k in range(KD):
                    nc.tensor.matmul(
                        pv[:],
                        lhsT=xts[k][:, xsl],
                        rhs=wv_sb[k][:],
                        start=(k == 0), stop=(k == KD - 1),
                    )
                vt = v_sb[nb * 4 + mt]
                vdst = vt[:].rearrange("p (h c) -> p h c", h=HL)[:, :, 0:HD]
                nc.vector.tensor_copy(vdst, pv[:].rearrange("p (h c) -> p h c", h=HL))
                ones_ap = vt[:].rearrange("p (h c) -> p h c", h=HL)[:, :, HD]
                nc.vector.memset(ones_ap, 1.0)

            fillers = []       # slow queue: spread over the phase
            fast_fillers = []  # fast queue: one per pair until drained
            quota = {"acc": 0.0, "rate": 1.0}

            def pop_filler():
                if fast_fillers:
                    fast_fillers.pop(0)()
                    return
                quota["acc"] += quota["rate"]
                while fillers and quota["acc"] >= 1.0:
                    quota["acc"] -= 1.0
                    fillers.pop(0)()

            def s2_head(h, qb):
                """Causal attention for head h, query block qb."""
                hp, parity = h // 2, h % 2
                rbase = 64 * parity
                qt = qk_sb[hp]
                kt = qk_sb[4 + hp]
                qsl0 = qb * QB
                o_ps = o_ps_pool.tile([128, 4 * VW], F32, tag="ops", name="o_ps")
                njp = 2 * qb + 2

                def issue_st(jp):
                    st = st_pool.tile([128, 1024], F32, tag="st", name="st")
                    cc0 = max(2 * jp - 4 * qb, 0) * TT
                    for half in (0, 1):
                        j = 2 * jp + half
                        nc.tensor.matmul(
                            st[:, half * QB + cc0:(half + 1) * QB],
                            lhsT=kt[rbase:rbase + HD, j * TT:(j + 1) * TT],
                            rhs=qt[rbase:rbase + HD, qsl0 + cc0:qsl0 + QB],
                            start=True, stop=True,
                        )
                    return st, cc0

                st_cur = issue_st(0)
                for jp in range(njp):
                    st, cc0 = st_cur
                    pt = pt_pool.tile([128, 1024], BF16, tag="pt", name="pt")
                    nc.scalar.activation(pt[:, cc0:1024], st[:, cc0:1024],
                                         AF.Exp, scale=1.0 / math.sqrt(HD))
                    st_cur = issue_st(jp + 1) if jp + 1 < njp else None
                    pop_filler()
                    for half in (0, 1):
                        j = 2 * jp + half
                        oi = j - 4 * qb
                        if oi >= 0:
                            dsl = slice(half * QB + oi * TT,
                                        half * QB + (oi + 1) * TT)
                            nc.vector.tensor_tensor(
                                pt[:, dsl], pt[:, dsl], trimask_sb[:],
                                op=ALU.mult)
                    for half in (0, 1):
                        j = 2 * jp + half
                        oi = j - 4 * qb
                        for qc in range(max(oi, 0), 4):
                            nc.tensor.matmul(
                                o_ps[:, qc * VW:(qc + 1) * VW],
                                lhsT=pt[:, half * QB + qc * TT:
                                        half * QB + (qc + 1) * TT],
                                rhs=v_sb[j][:, VW * h:VW * h + VW],
                                start=(j == 0 and qc == 0),
                                stop=(j == 4 * qb + qc),
                            )
                # normalize: o / denom, written into o_pair tiles (bf16)
                rec = rec_pool.tile([128, 4], F32, tag="rec", name="rec")
                dens = o_ps[:].rearrange("p (q c) -> p q c", q=4)[:, :, HD]
                nc.vector.reciprocal(rec[:], dens)
                return o_ps, rec

            opair_tiles = {}

            def s2_norm(h, qb, o_ps, rec):
                hp, parity = h // 2, h % 2
                key = (hp, qb)
                if key not in opair_tiles:
                    if qb < 3:
                        opair_tiles[key] = opair_pool.tile(
                            [128, QB], BF16, tag="opair", name="opair")
                    else:
                        opair_tiles[key] = opair_pool.tile(
                            [128, QB], F32, tag="opair3", name="opair3")
                opt = opair_tiles[key]
                for qc in range(4):
                    nc.vector.tensor_scalar(
                        opt[:, qc * TT + 64 * parity:qc * TT + 64 * parity + HD],
                        o_ps[:, qc * VW:qc * VW + HD],
                        rec[:, qc:qc + 1], None, op0=ALU.mult)

            def s2_transpose(hp, qb):
                """Transpose the finished o_pair row into oT_sb[hp].

                qb<3 uses the xbar DMA transpose; its consumers run a phase
                later, far beyond the transpose's completion.  qb==3 feeds
                the epilogue within ~1us, and the xbar transpose's
                completion semaphore can fire before all 16 engine chunks
                land, so the epilogue uses PE transposes with exact
                semaphore tracking instead.
                """
                opt = opair_tiles.pop((hp, qb))
                if qb < 3:
                    dst = oT_sb[hp][:, qb * QB:(qb + 1) * QB].rearrange(
                        "p (b q) -> p b q", b=4)
                    nc.sync.dma_start(dst, opt[:], transpose=True)
                    return
                for qc in range(4):
                    qtile = 4 * qb + qc
                    tp_t = half_pool.tile([128, QB], F32, tag="half",
                                          name="tp")
                    nc.tensor.transpose(
                        tp_t[:, 0:128], opt[:, qc * TT:(qc + 1) * TT],
                        identf_sb[:])
                    nc.vector.tensor_copy(
                        oT_sb[hp][:, qtile * TT:(qtile + 1) * TT],
                        tp_t[:, 0:128])

            def s3_group(mtt, ib, epi=False):
                tsl = slice(mtt * 128, (mtt + 1) * 128)
                if epi:
                    pot = st_pool.tile([128, 1024], F32, tag="st", name="po_e")
                    po = pot[:, 0:QB]
                else:
                    po = half_pool.tile([128, QB], F32, tag="half", name="po")
                for hp in range(4):
                    nc.tensor.matmul(
                        po[:],
                        lhsT=oT_sb[hp][:, tsl],
                        rhs=wout_sb[hp][:, ib * 512:(ib + 1) * 512],
                        start=(hp == 0), stop=(hp == 3),
                    )
                ot = out_pool.tile([128, QB], F32, tag="ot", name="ot")
                # epilogue halves alternate Act/DVE so the final copy+store
                # chains of the last token tiles pipeline two-wide
                if epi and ib == 0:
                    nc.scalar.copy(ot[:], po[:])
                else:
                    nc.vector.tensor_copy(ot[:], po[:])
                nc.sync.dma_start(outp[tsl, ib * 512:(ib + 1) * 512], ot[:])

            # ---------------- schedule ----------------
            # stage1 block 0 fully first (prologue)
            for mo in range(8):
                s1_qk(0, mo, xts_cur)
            for mt in range(4):
                s1_v(0, mt, xts_cur)

            def F(fn, *args):
                return lambda: fn(*args)

            s3_todo = []   # deferred stage-3 groups from previous qb
            xts_last = None
            for qb in range(nQB):
                xts_next = load_x(qb + 1) if qb + 1 < nQB else None
                if qb == 2:
                    xts_last = xts_next
                for hp in range(4):
                    for parity in (0, 1):
                        h = 2 * hp + parity
                        o_ps, rec = s2_head(h, qb)
                        s2_norm(h, qb, o_ps, rec)
                    s2_transpose(hp, qb)
                    if xts_next is not None:
                        s1_qk(qb + 1, 2 * hp, xts_next)
                        s1_qk(qb + 1, 2 * hp + 1, xts_next)
                        if hp % 2 == 1:
                            s1_v(qb + 1, hp - 1, xts_next)
                            s1_v(qb + 1, hp, xts_next)
                    n_s3 = {0: 0, 1: 1, 2: 2, 3: 6}[qb]
                    for _ in range(n_s3):
                        if s3_todo:
                            s3_group(*s3_todo.pop(0))
                for mtt in range(4 * qb, 4 * qb + 4):
                    for ib in range(2):
                        s3_todo.append((mtt, ib))
            epi = True
            while s3_todo:
                mtt, ib = s3_todo.pop(0)
                s3_group(mtt, ib, epi=epi)
                epi = not epi
            if debug:
                for i in range(8):
                    nc.sync.dma_start(qk_dbg[i * 128:(i + 1) * 128, :], qk_sb[i][:])
                for i in range(16):
                    nc.sync.dma_start(v_dbg[i * 128:(i + 1) * 128, :], v_sb[i][:])
                for i in range(4):
                    nc.sync.dma_start(oT_dbg[i * 128:(i + 1) * 128, :], oT_sb[i][:])

    nc.compile()
    return nc


# ---------------------------------------------------------------------------
# host side
# ---------------------------------------------------------------------------

_cache = {}


def _get_nc(S):
    if S not in _cache:
        _cache[S] = build_nc(S)
    return _cache[S]


def _rope_perm():
    """Row permutation within a 64-dim head: r = 2i+p  <-  i + 32p."""
    perm = np.empty(64, dtype=np.int64)
    for i in range(32):
        for p in (0, 1):
            perm[2 * i + p] = i + 32 * p
    return perm


def _shard_weights(w_qkv, w_out, g):
    """Per-head-group weight shards in device layouts (bf16)."""
    w_qkv = np.asarray(w_qkv, dtype=np.float32)
    w_out = np.asarray(w_out, dtype=np.float32)
    perm = _rope_perm()
    rows = []
    for part in range(2):           # 0: q, 1: k
        base = part * D
        for hl in range(HL):
            h_glob = g * HL + hl
            blk = w_qkv[base + h_glob * HD: base + (h_glob + 1) * HD]
            rows.append(blk[perm])
    wqk = np.concatenate(rows, axis=0)                 # [1024, 1024]
    wqkT = np.ascontiguousarray(wqk.T).astype(BF)      # [D, 1024]

    r = slice(2 * D + g * 512, 2 * D + (g + 1) * 512)
    wv = w_qkv[r]                                      # [512, 1024]
    wvT = np.ascontiguousarray(wv.T).astype(BF)        # [D, 512]

    woutT = np.ascontiguousarray(
        w_out.T[g * 512:(g + 1) * 512]).astype(BF)     # [512, 1024]
    return wqkT, wvT, woutT


def kernel(x, w_qkv, w_out):
    x = np.asarray(x, dtype=np.float32)
    B, S, _D = x.shape
    assert _D == D
    nc = _get_nc(S)

    shards = [_shard_weights(w_qkv, w_out, g) for g in range(2)]
    in_maps = []
    for core in range(8):
        b, g = core // 2, core % 2
        wqkT, wvT, woutT = shards[g]
        in_maps.append({
            "xT": np.ascontiguousarray(x[b].T).astype(BF),
            "wqkT": wqkT,
            "wvT": wvT,
            "woutT": woutT,
        })
    res = run_bass_kernel_spmd(nc, in_maps, list(range(8)))
    out = np.empty((B, S, D), dtype=np.float32)
    for b in range(B):
        out[b] = res.results[2 * b]["outp"] + res.results[2 * b + 1]["outp"]
    return out


# revision 7
# speedup vs baseline: 1.0082x; 1.0008x over previous
"""Multi-head causal attention (B=4, S=2048, D=1024, H=16, RoPE) on 8 TRN2 cores.

v2: bf16 compute, token-major PV with ones-column denominators, stream-shuffle
RoPE (pair-interleaved head dims), wide softmax exps, cross-stage interleaving.

Sharding: core = (batch b, head-group g of 8 heads).  Each core computes
qkv projection for its (b, g), RoPE, causal attention, and a partial
out-projection (contraction over its 512 head-dims).  Host sums the two
partials per batch.

Device layouts (per core):
  qk_sb[mo] [128, S]  feature-major bf16; mo 0..3 = q head pairs, 4..7 = k.
                      rows: head-local u*64 + 2*i + p  <->  orig dim i + 32p
                      (pair-interleaved so RoPE rotate-half is an adjacent-pair
                      partition swap, done by stream_shuffle within quadrants)
  v_sb[j]   [128, 520] token-major bf16, 65 cols/head: 64 dims + ones column
                      that makes the P@V matmul accumulate softmax denominators
  st        [t, q]    scores transposed (k-major) so they feed PV as lhsT
  o_ps      [q, 65]   per-(head,q-chunk) attention out + denominator column
  oT_sb[hp] [128, S]  transposed normalized outputs (PE transpose), out-proj lhsT
"""

import math

import numpy as np
import ml_dtypes

import concourse.bass as bass
import concourse.bacc as bacc
import concourse.mybir as mybir
from concourse import tile
from concourse.bass_utils import run_bass_kernel_spmd

AF = mybir.ActivationFunctionType
ALU = mybir.AluOpType
F32 = mybir.dt.float32
BF16 = mybir.dt.bfloat16

N_HEADS = 16
THETA = 10000.0
D = 1024
HD = 64
HL = 8          # heads per core
VW = HD + 1     # v cols per head (64 dims + ones)
QB = 512        # query block / stage-1 token block
TT = 128        # key/value tile
BF = ml_dtypes.bfloat16

SWAP_MASK = []
for _i in range(16):
    SWAP_MASK += [2 * _i + 1, 2 * _i]


def _host_constants(S):
    """RoPE tables (pair-interleaved rows), sign vector, triangle mask."""
    half = HD // 2
    inv = 1.0 / (THETA ** (np.arange(half, dtype=np.float64) / half))
    t = np.arange(S, dtype=np.float64)
    ang = inv[:, None] * t[None, :]                      # [32, S]
    cos = np.cos(ang)
    sin = np.sin(ang)
    # rows: u*64 + 2i + p  -> freq i (independent of u, p)
    cosr = np.repeat(cos, 2, axis=0)                     # [64, S]
    sinr = np.repeat(sin, 2, axis=0)
    ropeC = np.tile(cosr, (2, 1)).astype(BF)             # [128, S]
    sinT = np.tile(sinr, (2, 1)).astype(BF)
    sig = np.tile(np.array([-1.0, 1.0]), 64).astype(np.float32)[:, None]  # [128,1]
    p = np.arange(128)[:, None]
    c = np.arange(128)[None, :]
    trimask = (c >= p).astype(BF)                        # [128, 128]
    ident = np.eye(128, dtype=BF)
    return ropeC, sinT, sig, trimask, ident


def build_nc(S=2048, debug=False):
    nc = bacc.Bacc("TRN2", target_bir_lowering=False, debug=False)

    xT = nc.dram_tensor("xT", [D, S], BF16, kind="ExternalInput").ap()
    wqkT = nc.dram_tensor("wqkT", [D, 2 * HL * HD], BF16, kind="ExternalInput").ap()
    wvT = nc.dram_tensor("wvT", [D, HL * HD], BF16, kind="ExternalInput").ap()
    woutT = nc.dram_tensor("woutT", [HL * HD, D], BF16, kind="ExternalInput").ap()
    outp = nc.dram_tensor("outp", [S, D], F32, kind="ExternalOutput").ap()
    if debug:
        qk_dbg = nc.dram_tensor("qk_dbg", [8 * 128, S], BF16, kind="ExternalOutput").ap()
        v_dbg = nc.dram_tensor("v_dbg", [16 * 128, HL * VW], BF16, kind="ExternalOutput").ap()
        oT_dbg = nc.dram_tensor("oT_dbg", [4 * 128, S], BF16, kind="ExternalOutput").ap()

    ropeC_np, sinT_np, sig_np, trimask_np, ident_np = _host_constants(S)
    tabs_np = np.concatenate([ropeC_np, sinT_np, trimask_np, ident_np], axis=1)
    tabs_d = nc.inline_tensor(tabs_np, "tabs").ap()
    sig_d = nc.inline_tensor(sig_np, "sig").ap()
    identf_d = nc.inline_tensor(ident_np.astype(np.float32), "identf").ap()

    KD = D // 128        # 8 contraction tiles
    nQB = S // QB        # 4
    nMT = S // TT        # 16

    with tile.TileContext(nc) as tc:
        with (
            tc.tile_pool(name="qk", bufs=1) as qk_pool,
            tc.tile_pool(name="vres", bufs=1) as v_pool,
            tc.tile_pool(name="osb", bufs=1) as o_pool,
            tc.tile_pool(name="wqk", bufs=1) as wqk_pool,
            tc.tile_pool(name="wv", bufs=1) as wv_pool,
            tc.tile_pool(name="wout", bufs=1) as wout_pool,
            tc.tile_pool(name="tabs", bufs=1) as tab_pool,
            tc.tile_pool(name="xs", bufs=2) as x_pool,
            tc.tile_pool(name="rope", bufs=4) as rope_pool,
            tc.tile_pool(name="pt", bufs=4) as pt_pool,
            tc.tile_pool(name="opair", bufs=4) as opair_pool,
            tc.tile_pool(name="rec", bufs=4) as rec_pool,
            tc.tile_pool(name="outs", bufs=6) as out_pool,
            tc.tile_pool(name="ps_half", bufs=2, space="PSUM") as half_pool,
            tc.tile_pool(name="ps_st", bufs=2, space="PSUM") as st_pool,
            tc.tile_pool(name="ps_o", bufs=2, space="PSUM") as o_ps_pool,
        ):
            qk_sb = [qk_pool.tile([128, S], BF16, tag=f"qk{i}", name=f"qk{i}")
                     for i in range(8)]
            v_sb = [v_pool.tile([128, HL * VW], BF16, tag=f"v{i}", name=f"v{i}")
                    for i in range(nMT)]
            oT_sb = [o_pool.tile([128, S], BF16, tag=f"oT{i}", name=f"oT{i}")
                     for i in range(4)]
            wqk_big = wqk_pool.tile([128, KD * 128 * 8], BF16, tag="wqk",
                                    name="wqk_big")
            wv_big = wv_pool.tile([128, KD * HL * HD], BF16, tag="wv",
                                  name="wv_big")
            wout_big = wout_pool.tile([128, 4 * D], BF16, tag="wo",
                                      name="wout_big")
            wqk_sb = [wqk_big[:, k * 1024:(k + 1) * 1024] for k in range(KD)]
            wv_sb = [wv_big[:, k * 512:(k + 1) * 512] for k in range(KD)]
            wout_sb = [wout_big[:, i * D:(i + 1) * D] for i in range(4)]
            tabs_sb = tab_pool.tile([128, 2 * S + 256], BF16, tag="tabs",
                                    name="tabs")
            ropeC_sb = tabs_sb[:, 0:S]
            sinT_sb = tabs_sb[:, S:2 * S]
            trimask_sb = tabs_sb[:, 2 * S:2 * S + 128]
            ident_sb = tabs_sb[:, 2 * S + 128:2 * S + 256]
            sig_sb = tab_pool.tile([128, 1], F32, tag="sig", name="sig")
            identf_sb = tab_pool.tile([128, 128], F32, tag="identf",
                                      name="identf")

            # ---------------- prologue DMAs (one big DMA per tensor) -------
            def load_x(nb):
                tk = slice(nb * QB, (nb + 1) * QB)
                xt = x_pool.tile([128, KD * QB], BF16, tag="xt", name="xt")
                nc.sync.dma_start(
                    xt[:].rearrange("p (k c) -> p k c", k=KD),
                    xT[:, tk].rearrange("(k p) c -> p k c", p=128))
                return [xt[:, k * QB:(k + 1) * QB] for k in range(KD)]

            xt0 = x_pool.tile([128, KD * QB], BF16, tag="xt", name="xt")
            for lo in range(0, KD, 2):
                nc.sync.dma_start(
                    wqk_big[:, lo * 1024:(lo + 2) * 1024].rearrange(
                        "p (k c) -> p k c", k=2),
                    wqkT[lo * 128:(lo + 2) * 128, :].rearrange(
                        "(k p) c -> p k c", p=128))
                nc.sync.dma_start(
                    xt0[:, lo * QB:(lo + 2) * QB].rearrange(
                        "p (k c) -> p k c", k=2),
                    xT[lo * 128:(lo + 2) * 128, 0:QB].rearrange(
                        "(k p) c -> p k c", p=128))
            xts0 = [xt0[:, k * QB:(k + 1) * QB] for k in range(KD)]
            nc.sync.dma_start(tabs_sb[:], tabs_d[:])
            nc.sync.dma_start(sig_sb[:], sig_d[:])
            nc.sync.dma_start(identf_sb[:], identf_d[:])
            nc.sync.dma_start(
                wv_big[:].rearrange("p (k c) -> p k c", k=KD),
                wvT.rearrange("(k p) c -> p k c", p=128))
            nc.sync.dma_start(
                wout_big[:].rearrange("p (k c) -> p k c", k=4),
                woutT.rearrange("(k p) c -> p k c", p=128))

            xts_cur = xts0

            # ---------------- stage pieces ----------------
            st_half = {"tile": None}

            def deep_ps(mo):
                """Prologue-only: 6-deep PSUM ring borrowing idle st banks."""
                sel = mo % 3
                if sel < 2:
                    if sel == 0:
                        st_half["tile"] = st_pool.tile(
                            [128, 1024], F32, tag="st", name="ps_qk_st")
                        return st_half["tile"][:, 0:QB]
                    return st_half["tile"][:, QB:2 * QB]
                return half_pool.tile([128, QB], F32, tag="half", name="ps_qk")

            def s1_qk(nb, mo, xts):
                """q/k projection tile mo for token block nb, with RoPE."""
                tok = slice(nb * QB, (nb + 1) * QB)
                ps = half_pool.tile([128, QB], F32, tag="half", name="ps_qk")
                for k in range(KD):
                    nc.tensor.matmul(
                        ps[:],
                        lhsT=wqk_sb[k][:, mo * 128:(mo + 1) * 128],
                        rhs=xts[k][:],
                        start=(k == 0), stop=(k == KD - 1),
                    )
                # RoPE: qk = ps*cos + pairswap(ps)*sig*sin
                swp = rope_pool.tile([128, QB], F32, tag="swp", name="swp")
                nc.vector.stream_shuffle(swp[:], ps[:], SWAP_MASK)
                cq = rope_pool.tile([128, QB], BF16, tag="cq", name="cq")
                nc.vector.scalar_tensor_tensor(
                    cq[:], ps[:], 1.0, ropeC_sb[:, tok],
                    op0=ALU.mult, op1=ALU.mult,
                )
                tm = rope_pool.tile([128, QB], BF16, tag="tm", name="tm")
                nc.vector.scalar_tensor_tensor(
                    tm[:], swp[:], sig_sb[:, 0:1], sinT_sb[:, tok],
                    op0=ALU.mult, op1=ALU.mult,
                )
                nc.gpsimd.tensor_tensor(
                    qk_sb[mo][:, tok], cq[:], tm[:], op=ALU.add)

            def s1_v(nb, mt, xts):
                """v projection for token tile nb*4+mt (token-major + ones)."""
                pv = half_pool.tile([128, QB], F32, tag="half", name="ps_v")
                xsl = slice(mt * 128, (mt + 1) * 128)
                for k in range(KD):
                    nc.tensor.matmul(
                        pv[:],
                        lhsT=xts[k][:, xsl],
                        rhs=wv_sb[k][:],
                        start=(k == 0), stop=(k == KD - 1),
                    )
                vt = v_sb[nb * 4 + mt]
                vdst = vt[:].rearrange("p (h c) -> p h c", h=HL)[:, :, 0:HD]
                nc.vector.tensor_copy(vdst, pv[:].rearrange("p (h c) -> p h c", h=HL))
                ones_ap = vt[:].rearrange("p (h c) -> p h c", h=HL)[:, :, HD]
                nc.vector.memset(ones_ap, 1.0)

            fillers = []       # slow queue: spread over the phase
            fast_fillers = []  # fast queue: one per pair until drained
            quota = {"acc": 0.0, "rate": 1.0}

            def pop_filler():
                if fast_fillers:
                    fast_fillers.pop(0)()
                    return
                quota["acc"] += quota["rate"]
                while fillers and quota["acc"] >= 1.0:
                    quota["acc"] -= 1.0
                    fillers.pop(0)()

            def s2_head(h, qb):
                """Causal attention for head h, query block qb."""
                hp, parity = h // 2, h % 2
                rbase = 64 * parity
                qt = qk_sb[hp]
                kt = qk_sb[4 + hp]
                qsl0 = qb * QB
                o_ps = o_ps_pool.tile([128, 4 * VW], F32, tag="ops", name="o_ps")
                njp = 2 * qb + 2

                def issue_st(jp):
                    st = st_pool.tile([128, 1024], F32, tag="st", name="st")
                    cc0 = max(2 * jp - 4 * qb, 0) * TT
                    for half in (0, 1):
                        j = 2 * jp + half
                        nc.tensor.matmul(
                            st[:, half * QB + cc0:(half + 1) * QB],
                            lhsT=kt[rbase:rbase + HD, j * TT:(j + 1) * TT],
                            rhs=qt[rbase:rbase + HD, qsl0 + cc0:qsl0 + QB],
                            start=True, stop=True,
                        )
                    return st, cc0

                st_cur = issue_st(0)
                for jp in range(njp):
                    st, cc0 = st_cur
                    pt = pt_pool.tile([128, 1024], BF16, tag="pt", name="pt")
                    nc.scalar.activation(pt[:, cc0:1024], st[:, cc0:1024],
                                         AF.Exp, scale=1.0 / math.sqrt(HD))
                    st_cur = issue_st(jp + 1) if jp + 1 < njp else None
                    pop_filler()
                    for half in (0, 1):
                        j = 2 * jp + half
                        oi = j - 4 * qb
                        if oi >= 0:
                            dsl = slice(half * QB + oi * TT,
                                        half * QB + (oi + 1) * TT)
                            nc.vector.tensor_tensor(
                                pt[:, dsl], pt[:, dsl], trimask_sb[:],
                                op=ALU.mult)
                    for half in (0, 1):
                        j = 2 * jp + half
                        oi = j - 4 * qb
                        for qc in range(max(oi, 0), 4):
                            nc.tensor.matmul(
                                o_ps[:, qc * VW:(qc + 1) * VW],
                                lhsT=pt[:, half * QB + qc * TT:
                                        half * QB + (qc + 1) * TT],
                                rhs=v_sb[j][:, VW * h:VW * h + VW],
                                start=(j == 0 and qc == 0),
                                stop=(j == 4 * qb + qc),
                            )
                # normalize: o / denom, written into o_pair tiles (bf16)
                rec = rec_pool.tile([128, 4], F32, tag="rec", name="rec")
                dens = o_ps[:].rearrange("p (q c) -> p q c", q=4)[:, :, HD]
                nc.vector.reciprocal(rec[:], dens)
                return o_ps, rec

            opair_tiles = {}

            def s2_norm(h, qb, o_ps, rec):
                hp, parity = h // 2, h % 2
                key = (hp, qb)
                if key not in opair_tiles:
                    if qb < 3:
                        opair_tiles[key] = opair_pool.tile(
                            [128, QB], BF16, tag="opair", name="opair")
                    else:
                        opair_tiles[key] = opair_pool.tile(
                            [128, QB], F32, tag="opair3", name="opair3")
                opt = opair_tiles[key]
                for qc in range(4):
                    nc.vector.tensor_scalar(
                        opt[:, qc * TT + 64 * parity:qc * TT + 64 * parity + HD],
                        o_ps[:, qc * VW:qc * VW + HD],
                        rec[:, qc:qc + 1], None, op0=ALU.mult)

            def s2_transpose(hp, qb):
                """Transpose the finished o_pair row into oT_sb[hp].

                qb<3 uses the xbar DMA transpose; its consumers run a phase
                later, far beyond the transpose's completion.  qb==3 feeds
                the epilogue within ~1us, and the xbar transpose's
                completion semaphore can fire before all 16 engine chunks
                land, so the epilogue uses PE transposes with exact
                semaphore tracking instead.
                """
                opt = opair_tiles.pop((hp, qb))
                if qb < 3:
                    dst = oT_sb[hp][:, qb * QB:(qb + 1) * QB].rearrange(
                        "p (b q) -> p b q", b=4)
                    nc.sync.dma_start(dst, opt[:], transpose=True)
                    return
                for qc in range(4):
                    qtile = 4 * qb + qc
                    tp_t = half_pool.tile([128, QB], F32, tag="half",
                                          name="tp")
                    nc.tensor.transpose(
                        tp_t[:, 0:128], opt[:, qc * TT:(qc + 1) * TT],
                        identf_sb[:])
                    nc.vector.tensor_copy(
                        oT_sb[hp][:, qtile * TT:(qtile + 1) * TT],
                        tp_t[:, 0:128])

            def s3_group(mtt, ib, epi=False):
                tsl = slice(mtt * 128, (mtt + 1) * 128)
                if epi:
                    pot = st_pool.tile([128, 1024], F32, tag="st", name="po_e")
                    po = pot[:, 0:QB]
                else:
                    po = half_pool.tile([128, QB], F32, tag="half", name="po")
                for hp in range(4):
                    nc.tensor.matmul(
                        po[:],
                        lhsT=oT_sb[hp][:, tsl],
                        rhs=wout_sb[hp][:, ib * 512:(ib + 1) * 512],
                        start=(hp == 0), stop=(hp == 3),
                    )
                ot = out_pool.tile([128, QB], F32, tag="ot", name="ot")
                # epilogue halves alternate Act/DVE so the final copy+store
                # chains of the last token tiles pipeline two-wide
                if epi and ib == 0:
                    nc.scalar.copy(ot[:], po[:])
                else:
                    nc.vector.tensor_copy(ot[:], po[:])
                nc.sync.dma_start(outp[tsl, ib * 512:(ib + 1) * 512], ot[:])

            # ---------------- schedule ----------------
            # stage1 block 0 fully first (prologue)
            for mo in range(8):
                s1_qk(0, mo, xts_cur)
            for mt in range(4):
                s1_v(0, mt, xts_cur)

            def F(fn, *args):
                return lambda: fn(*args)

            s3_todo = []   # deferred stage-3 groups from previous qb
            xts_last = None
            for qb in range(nQB):
                xts_next = load_x(qb + 1) if qb + 1 < nQB else None
                if qb == 2:
                    xts_last = xts_next
                for hp in range(4):
                    for parity in (0, 1):
                        h = 2 * hp + parity
                        o_ps, rec = s2_head(h, qb)
                        s2_norm(h, qb, o_ps, rec)
                    s2_transpose(hp, qb)
                    if xts_next is not None:
                        s1_qk(qb + 1, 2 * hp, xts_next)
                        s1_qk(qb + 1, 2 * hp + 1, xts_next)
                        if hp % 2 == 1:
                            s1_v(qb + 1, hp - 1, xts_next)
                            s1_v(qb + 1, hp, xts_next)
                    n_s3 = {0: 0, 1: 1, 2: 2, 3: 6}[qb]
                    for _ in range(n_s3):
                        if s3_todo:
                            s3_group(*s3_todo.pop(0))
                for mtt in range(4 * qb, 4 * qb + 4):
                    for ib in range(2):
                        s3_todo.append((mtt, ib))
            epi = True
            while s3_todo:
                mtt, ib = s3_todo.pop(0)
                s3_group(mtt, ib, epi=epi)
                epi = not epi
            if debug:
                for i in range(8):
                    nc.sync.dma_start(qk_dbg[i * 128:(i + 1) * 128, :], qk_sb[i][:])
                for i in range(16):
                    nc.sync.dma_start(v_dbg[i * 128:(i + 1) * 128, :], v_sb[i][:])
                for i in range(4):
                    nc.sync.dma_start(oT_dbg[i * 128:(i + 1) * 128, :], oT_sb[i][:])

    nc.compile()
    return nc


# ---------------------------------------------------------------------------
# host side
# ---------------------------------------------------------------------------

_cache = {}


def _get_nc(S):
    if S not in _cache:
        _cache[S] = build_nc(S)
    return _cache[S]


def _rope_perm():
    """Row permutation within a 64-dim head: r = 2i+p  <-  i + 32p."""
    perm = np.empty(64, dtype=np.int64)
    for i in range(32):
        for p in (0, 1):
            perm[2 * i + p] = i + 32 * p
    return perm


def _shard_weights(w_qkv, w_out, g):
    """Per-head-group weight shards in device layouts (bf16)."""
    w_qkv = np.asarray(w_qkv, dtype=np.float32)
    w_out = np.asarray(w_out, dtype=np.float32)
    perm = _rope_perm()
    rows = []
    for part in range(2):           # 0: q, 1: k
        base = part * D
        for hl in range(HL):
            h_glob = g * HL + hl
            blk = w_qkv[base + h_glob * HD: base + (h_glob + 1) * HD]
            rows.append(blk[perm])
    wqk = np.concatenate(rows, axis=0)                 # [1024, 1024]
    wqkT = np.ascontiguousarray(wqk.T).astype(BF)      # [D, 1024]

    r = slice(2 * D + g * 512, 2 * D + (g + 1) * 512)
    wv = w_qkv[r]                                      # [512, 1024]
    wvT = np.ascontiguousarray(wv.T).astype(BF)        # [D, 512]

    woutT = np.ascontiguousarray(
        w_out.T[g * 512:(g + 1) * 512]).astype(BF)     # [512, 1024]
    return wqkT, wvT, woutT


def kernel(x, w_qkv, w_out):
    x = np.asarray(x, dtype=np.float32)
    B, S, _D = x.shape
    assert _D == D
    nc = _get_nc(S)

    shards = [_shard_weights(w_qkv, w_out, g) for g in range(2)]
    in_maps = []
    for core in range(8):
        b, g = core // 2, core % 2
        wqkT, wvT, woutT = shards[g]
        in_maps.append({
            "xT": np.ascontiguousarray(x[b].T).astype(BF),
            "wqkT": wqkT,
            "wvT": wvT,
            "woutT": woutT,
        })
    res = run_bass_kernel_spmd(nc, in_maps, list(range(8)))
    out = np.empty((B, S, D), dtype=np.float32)
    for b in range(B):
        out[b] = res.results[2 * b]["outp"] + res.results[2 * b + 1]["outp"]
    return out
